# revision 27
# baseline (speedup 1.0000x reference)
"""AttentionPooling Trainium2 kernel (v3: contiguous load + diagonal pooling).

Math (per batch row b):
    x   = target[b] + hist[b]              # [T, D]
    h   = relu(x @ W + Wb)                 # [T, D]
    lg  = h @ q (+ q_bias, softmax-invariant -> ignored)
    s   = softmax(lg)                      # over T
    out = sum_t s_t * hist[b, t]           # [D]

Device strategy (pure data parallel over batch across 8 cores).  Strided
HBM reads run at ~half bandwidth on real TRN2, so hist is loaded with a
single fully CONTIGUOUS fp32->bf16 cast DMA per 64-batch iteration into
the natural layout [p=(b,th), (tl,d)] (th = t//100, tl = t%100):
  - PE transposes the 100 [128,128] d-blocks -> xT [d, (tl, p)]; the
    PSUM->SBUF copy fuses the broadcast target add (packed APs keep the
    DVE 2x bf16 mode).
  - Main matmul: H^T = W^T @ xT (bf16, W stationary), relu+bias on ACT
    over 1024-col PSUM chunks.
  - q-matmul per tl-chunk: stationary = hh block [e,128], moving = q
    -> logits land NATURALLY as [p, tl] columns of one [128,100] PSUM
    tile; a single exp (ACT) with accum_out yields w AND the softmax
    denominators in one instruction.
  - Pooling: per tl one matmul, stationary = wdiag [128, 64] (w values
    scattered on the 2-diagonal (p, p//2), built by one DVE multiply
    with a constant 0/1 mask), moving = the natural hist block
    [128, 128] -> PSUM-accumulated [b, d] over all 100 tl.
  - Final normalize (divide by sum_t w) on host.
"""

import sys

sys.path.insert(0, "/opt/trn_rl_repo")

import numpy as np

import concourse.bacc as bacc
import concourse.bass as bass
import concourse.mybir as mybir
import concourse.tile as tile
from concourse import masks
from concourse.bass_utils import run_bass_kernel_spmd

F32 = mybir.dt.float32
BF16 = mybir.dt.bfloat16
AF = mybir.ActivationFunctionType

NCORES = 8
B, T, D = 16384, 200, 128
BC = B // NCORES          # 2048 batch rows per core
TL = 100                  # tl positions per partition (t = th*100 + tl)
B_IT = 64                 # batch rows per outer iteration
NC_IT = B_IT * T * D      # elements per iteration


def build(nc, b_core=BC, dbg=False):
    nit = b_core // B_IT
    hist = nc.dram_tensor("hist", [b_core, T, D], F32, kind="ExternalInput")
    tgt = nc.dram_tensor("target", [b_core, D], F32, kind="ExternalInput")
    w_in = nc.dram_tensor("W", [D, D], F32, kind="ExternalInput")
    wb_in = nc.dram_tensor("Wb", [D], F32, kind="ExternalInput")
    q_in = nc.dram_tensor("q", [D, 1], F32, kind="ExternalInput")
    out_pl = nc.dram_tensor("out_pl", [nit, B_IT, D], F32, kind="ExternalOutput")
    out_dn = nc.dram_tensor("out_dn", [nit, 128, 2], F32, kind="ExternalOutput")
    if dbg:
        dbg_nt = nc.dram_tensor("dbg_nt", [128, TL * D], F32, kind="ExternalOutput")
        dbg_ht = nc.dram_tensor("dbg_ht", [128, B_IT * T], F32, kind="ExternalOutput")
        dbg_hh = nc.dram_tensor("dbg_hh", [128, B_IT * T], F32, kind="ExternalOutput")
        dbg_w = nc.dram_tensor("dbg_w", [128, TL], F32, kind="ExternalOutput")
        dbg_wd = nc.dram_tensor("dbg_wd", [128, TL * B_IT], F32, kind="ExternalOutput")

    from contextlib import ExitStack
    with tile.TileContext(nc) as tc, ExitStack() as es:
        consts = es.enter_context(tc.tile_pool(name="consts", bufs=1))
        nt_pool = es.enter_context(tc.tile_pool(name="nt", bufs=CFG["nt"]))
        ht_pool = es.enter_context(tc.tile_pool(name="ht", bufs=CFG["ht"]))
        h_pool = es.enter_context(tc.tile_pool(name="h", bufs=CFG["hh"]))
        w_pool = es.enter_context(tc.tile_pool(name="w", bufs=CFG["wb"]))
        out_pool = es.enter_context(tc.tile_pool(name="out", bufs=CFG["outt"]))
        ps_tp = es.enter_context(tc.tile_pool(name="ps_tp", bufs=CFG["tp"], space="PSUM"))
        ps_mm = es.enter_context(tc.tile_pool(name="ps_mm", bufs=CFG["mm"], space="PSUM"))
        ps_q = es.enter_context(tc.tile_pool(name="ps_q", bufs=CFG["q"], space="PSUM"))
        ps_pool = es.enter_context(tc.tile_pool(name="ps_pool", bufs=CFG["pool"], space="PSUM"))

        # ---- constants ----
        ident = consts.tile([128, 128], BF16)
        masks.make_identity(nc, ident[:, :])

        w_f32 = consts.tile([D, D], F32)
        nc.sync.dma_start(out=w_f32, in_=w_in.ap())
        w_bf = consts.tile([D, D], BF16)
        nc.vector.tensor_copy(out=w_bf, in_=w_f32)

        wbias = consts.tile([D, 1], F32)
        nc.sync.dma_start(out=wbias, in_=wb_in.ap()[:, None])

        q_f32 = consts.tile([D, 1], F32)
        nc.sync.dma_start(out=q_f32, in_=q_in.ap())
        q_bf = consts.tile([D, 1], BF16)
        nc.vector.tensor_copy(out=q_bf, in_=q_f32)

        # 2-diagonal mask: I2[p, b] = 1 if p // 2 == b else 0  [128, 64] bf16
        # built from the identity: I2[p, b] = ident[p, 2b] + ident[p, 2b+1]
        i2 = consts.tile([128, B_IT], BF16)
        idv = ident.rearrange("p (b u) -> p b u", u=2)
        nc.vector.tensor_add(i2, idv[:, :, 0], idv[:, :, 1])

        # targetT [d, b_core] bf16
        tgtT = consts.tile([D, b_core], BF16)
        for k in range((b_core + 127) // 128):
            bn = min(128, b_core - k * 128)
            t_f32 = w_pool.tile([128, D], F32, tag="tsetup")
            nc.sync.dma_start(out=t_f32[0:bn], in_=tgt.ap()[k * 128:k * 128 + bn, :])
            t_bf = w_pool.tile([128, D], BF16, tag="tsetup_bf")
            nc.vector.tensor_copy(out=t_bf[0:bn], in_=t_f32[0:bn])
            tpp = ps_tp.tile([128, 1024], BF16, tag="tp")
            nc.tensor.transpose(tpp[:, 0:bn], t_bf[0:bn], ident[0:bn, 0:bn])
            nc.vector.tensor_copy(out=tgtT[:, k * 128:k * 128 + bn], in_=tpp[:, 0:bn])

        # ---- main loop ----
        # Pooling for iteration it is emitted during iteration it+1 (after
        # the transposes) so the PE never stalls on the exp -> wdiag chain.
        def emit_pool(prev):
            it_p, nt_p, wd_p = prev
            pl = ps_pool.tile([B_IT, D], F32)
            for tl in range(TL):
                nc.tensor.matmul(pl, wd_p[:, tl * B_IT:(tl + 1) * B_IT],
                                 nt_p[:, tl * D:tl * D + D],
                                 start=tl == 0, stop=tl == TL - 1)
            outt = out_pool.tile([B_IT, D], F32, tag="outt")
            nc.vector.tensor_copy(out=outt, in_=pl)
            nc.sync.dma_start(out=out_pl.ap()[it_p], in_=outt)

        prev = None
        for it in range(nit):
            b0 = it * B_IT

            # natural tile: partition p=(b,th), free (tl, d); one contiguous
            # cast DMA for the whole 64-batch slice
            nt = nt_pool.tile([128, TL * D], BF16, tag="nt")
            src = hist.ap()[b0:b0 + B_IT]
            nc.gpsimd.dma_start(
                out=nt,
                in_=bass.AP(tensor=src.tensor, offset=src.offset,
                            ap=[[TL * D, 128], [1, TL * D]]),
            )

            # target expanded 2x: tgx2[d, p] = tgtT[d, b0 + p//2] -- i.e.
            # column index IS p = 2b+th, so the broadcast AP below has a
            # fully packed innermost dim (DVE 2x mode)
            tgx2 = w_pool.tile([128, B_IT * 2], BF16, tag="tgx2")
            sl = tgtT[:, b0:b0 + B_IT]
            nc.vector.tensor_copy(
                out=tgx2,
                in_=bass.AP(tensor=sl.tensor, offset=sl.offset,
                            ap=[sl.ap[0], sl.ap[1], [0, 2]]),
            )

            # transposes -> xT [d, (tl, p)] with fused target add
            ht = ht_pool.tile([128, B_IT * T], BF16, tag="ht")
            NTG = CFG["ntg"]          # transposes per PSUM group (8 -> 1 bank)
            for g in range((TL + NTG - 1) // NTG) if "tp" not in SKIP else []:
                t0 = NTG * g
                ng = min(NTG, TL - t0)
                tp = ps_tp.tile([128, NTG * 128], BF16, tag="tp")
                for u in range(ng):
                    nc.tensor.transpose(
                        tp[:, 128 * u:128 * u + 128],
                        nt[:, (t0 + u) * D:(t0 + u) * D + D], ident)
                nc.vector.tensor_add(
                    ht.rearrange("d (t p) -> d t p",
                                 p=128)[:, t0:t0 + ng, :],
                    tp.rearrange("d (t p) -> d t p", p=128)[:, 0:ng, :],
                    bass.AP(tensor=tgx2.tensor, offset=tgx2.offset,
                            ap=[tgx2.ap[0], [0, ng], [1, 128]]),
                )

            # H^T = relu(W^T xT + bias)  [e, (tl, p)]
            hh = h_pool.tile([128, B_IT * T], BF16, tag="hh")
            nmm = (B_IT * T) // 1024
            for k in range(nmm + 1) if "mm" not in SKIP else []:
                w_cols = 1024 if k < nmm else (B_IT * T) % 1024
                if w_cols == 0:
                    continue
                mm = ps_mm.tile([128, 1024], F32)
                for h in range((w_cols + 511) // 512):
                    c = 1024 * k + 512 * h
                    cw = min(512, w_cols - 512 * h)
                    nc.tensor.matmul(mm[:, 512 * h:512 * h + cw], w_bf,
                                     ht[:, c:c + cw], start=True, stop=True)
                nc.scalar.activation(hh[:, 1024 * k:1024 * k + w_cols],
                                     mm[:, 0:w_cols], AF.Relu, bias=wbias)

            # q-matmuls (logits land naturally [p, tl]) interleaved with the
            # previous iteration's pooling matmuls: the qn stationary loads
            # (128 cols, not overlappable behind qn's own 1-col streams)
            # hide under the pool matmuls' 128-col streams.
            qn = ps_q.tile([128, TL], F32)
            do_pool = prev is not None and "pool" not in SKIP
            if do_pool:
                it_p, nt_p, wd_p = prev
                pl = ps_pool.tile([B_IT, D], F32)
            for tl in range(TL) if "q" not in SKIP else []:
                nc.tensor.matmul(qn[:, tl:tl + 1],
                                 hh[:, tl * 128:tl * 128 + 128], q_bf,
                                 start=True, stop=True)
                if do_pool:
                    nc.tensor.matmul(pl, wd_p[:, tl * B_IT:(tl + 1) * B_IT],
                                     nt_p[:, tl * D:tl * D + D],
                                     start=tl == 0, stop=tl == TL - 1)
            if do_pool:
                outt = out_pool.tile([B_IT, D], F32, tag="outt")
                nc.vector.tensor_copy(out=outt, in_=pl)
                nc.sync.dma_start(out=out_pl.ap()[it_p], in_=outt)

            # exp in 2 chunks (pipelines with qn); accum gives denominators
            wnat = w_pool.tile([128, TL], BF16, tag="wnat")
            dn_sb = out_pool.tile([128, 2], F32, tag="dn")
            if "q" not in SKIP:
                for c in range(2):
                    nc.scalar.activation(wnat[:, 50 * c:50 * c + 50],
                                         qn[:, 50 * c:50 * c + 50], AF.Exp,
                                         accum_out=dn_sb[:, c:c + 1])
                nc.sync.dma_start(out=out_dn.ap()[it], in_=dn_sb)

            # wdiag build in 4 chunks: wdiag[p, (tl, b)] = I2[p, b] * wnat[p, tl]
            wdiag = w_pool.tile([128, TL * B_IT], BF16, tag="wdiag")
            wdv = wdiag.rearrange("p (t b) -> p t b", b=B_IT)
            if "pool" not in SKIP and "q" not in SKIP:
                for c in range(4):
                    wn = wnat[:, 25 * c:25 * c + 25]
                    nc.vector.tensor_mul(
                        wdv[:, 25 * c:25 * c + 25, :],
                        bass.AP(tensor=i2.tensor, offset=i2.offset,
                                ap=[i2.ap[0], [0, 25], i2.ap[1]]),
                        bass.AP(tensor=wn.tensor, offset=wn.offset,
                                ap=[wn.ap[0], wn.ap[1], [0, B_IT]]),
                    )
            prev = (it, nt, wdiag)

            if dbg and it == 0:
                nc.gpsimd.dma_start(out=dbg_nt.ap(), in_=nt)
                nc.gpsimd.dma_start(out=dbg_ht.ap(), in_=ht)
                nc.gpsimd.dma_start(out=dbg_hh.ap(), in_=hh)
                nc.gpsimd.dma_start(out=dbg_w.ap(), in_=wnat)
                nc.gpsimd.dma_start(out=dbg_wd.ap(), in_=wdiag)

        if prev is not None and "pool" not in SKIP:
            emit_pool(prev)

    return out_pl


def decode_out(pl, dn, b_core=BC):
    """out_pl [nit, B_IT, D], out_dn [nit, 128, 2] -> pooled, wsum per b."""
    nit = b_core // B_IT
    pooled = pl.reshape(b_core, D)
    d = dn.reshape(nit, B_IT, 2, 2).astype(np.float64)
    wsum = d.sum(axis=(2, 3)).reshape(b_core).astype(np.float32)
    return pooled, wsum


_cache = {}
LAST_RESULT = None
SKIP = set()
CFG = dict(nt=2, tp=2, mm=2, q=1, pool=1, ht=2, hh=1, outt=2, wb=3, ntg=8)


def _get_program(b_core):
    key = (b_core, tuple(sorted(SKIP)), tuple(sorted(CFG.items())))
    if key not in _cache:
        nc = bacc.Bacc("TRN2", target_bir_lowering=False, debug=False,
                       num_devices=NCORES)
        build(nc, b_core)
        nc.compile()
        _cache[key] = nc
    return _cache[key]


def kernel(**inputs):
    hist = np.ascontiguousarray(np.asarray(inputs["hist_embeddings"], np.float32))
    tgt = np.ascontiguousarray(np.asarray(inputs["target_embedding"], np.float32))
    W = np.ascontiguousarray(np.asarray(inputs["W_kernel"], np.float32))
    Wb = np.ascontiguousarray(np.asarray(inputs["W_bias"], np.float32))
    q = np.ascontiguousarray(np.asarray(inputs["q_kernel"], np.float32))
    # q_bias shifts every logit equally -> softmax-invariant -> ignored.

    nc = _get_program(BC)
    in_maps = []
    for c in range(NCORES):
        sl = slice(c * BC, (c + 1) * BC)
        in_maps.append({
            "hist": hist[sl], "target": tgt[sl],
            "W": W, "Wb": Wb, "q": q,
        })
    res = run_bass_kernel_spmd(nc, in_maps, core_ids=list(range(NCORES)))
    global LAST_RESULT
    LAST_RESULT = res
    outs = []
    for c in range(NCORES):
        pooled, wsum = decode_out(res.results[c]["out_pl"],
                                  res.results[c]["out_dn"])
        outs.append(pooled / wsum[:, None])
    return np.concatenate(outs, axis=0).astype(np.float32)


def timed_run(inputs, iters=5, bcs=BC):
    """Device-resident repeated execution; returns (best_seconds, outputs)."""
    import time
    import jax
    from jax.sharding import Mesh, PartitionSpec
    from jax.experimental.shard_map import shard_map
    import concourse.mybir as mybir_
    from concourse.bass2jax import (install_neuronx_cc_hook, _bass_exec_p,
                                    partition_id_tensor)

    hist = np.ascontiguousarray(np.asarray(inputs["hist_embeddings"], np.float32))
    tgt = np.ascontiguousarray(np.asarray(inputs["target_embedding"], np.float32))
    W = np.ascontiguousarray(np.asarray(inputs["W_kernel"], np.float32))
    Wb = np.ascontiguousarray(np.asarray(inputs["W_bias"], np.float32))
    q = np.ascontiguousarray(np.asarray(inputs["q_kernel"], np.float32))
    hist = hist[:NCORES * bcs].reshape(NCORES * bcs, T, D)
    tgt = tgt[:NCORES * bcs]
    nc = _get_program(bcs)
    install_neuronx_cc_hook()

    pid_name = nc.partition_id_tensor.name if nc.partition_id_tensor else None
    in_names, out_names, out_avals, zero_outs = [], [], [], []
    for alloc in nc.m.functions[0].allocations:
        if not isinstance(alloc, mybir_.MemoryLocationSet):
            continue
        name = alloc.memorylocations[0].name
        if alloc.kind == "ExternalInput":
            if name != pid_name:
                in_names.append(name)
        elif alloc.kind == "ExternalOutput":
            shape = tuple(alloc.tensor_shape)
            dtype = mybir_.dt.np(alloc.dtype)
            out_names.append(name)
            out_avals.append(jax.core.ShapedArray(shape, dtype))
            zero_outs.append(np.zeros(shape, dtype))
    all_names = in_names + out_names
    if pid_name is not None:
        all_names = all_names + [pid_name]

    import os
    chain = int(os.environ.get("KERNEL_CHAIN", "1"))

    def _body(*args):
        nin_ = len(in_names)
        ins_ = list(args[:nin_])
        outs = list(args[nin_:])
        for _ in range(chain):
            operands = ins_ + outs
            if pid_name is not None:
                operands = operands + [partition_id_tensor()]
            outs = list(_bass_exec_p.bind(
                *operands, out_avals=tuple(out_avals),
                in_names=tuple(all_names), out_names=tuple(out_names),
                lowering_input_output_aliases=(),
                sim_require_finite=True, sim_require_nnan=True, nc=nc))
        return tuple(outs)

    devices = jax.devices()[:NCORES]
    mesh = Mesh(np.array(devices), ("core",))
    nin = len(in_names) + len(out_names)
    fn = jax.jit(shard_map(_body, mesh=mesh,
                           in_specs=(PartitionSpec("core"),) * nin,
                           out_specs=(PartitionSpec("core"),) * len(out_names),
                           check_rep=False))
    full = {"hist": hist, "target": tgt,
            "W": np.concatenate([W] * NCORES, 0),
            "Wb": np.concatenate([Wb] * NCORES, 0),
            "q": np.concatenate([q] * NCORES, 0)}
    args = [full[n] for n in in_names] + [
        np.concatenate([z] * NCORES, 0) for z in zero_outs]
    sh = jax.sharding.NamedSharding(mesh, PartitionSpec("core"))
    dargs = [jax.device_put(a, sh) for a in args]
    res = fn(*dargs)
    jax.block_until_ready(res)
    import os
    pipeline = int(os.environ.get("KERNEL_PIPE", "1"))
    nin_ = len(in_names)
    best = float("inf")
    for _ in range(iters):
        t0 = time.perf_counter()
        r = tuple(dargs[nin_:])
        for _k in range(pipeline):
            r = fn(*dargs[:nin_], *r)
        jax.block_until_ready(r)
        best = min(best, time.perf_counter() - t0)
        res = r
    outs = [np.asarray(r) for r in res]
    pl_all = np.split(outs[out_names.index("out_pl")], NCORES, axis=0)
    dn_all = np.split(outs[out_names.index("out_dn")], NCORES, axis=0)
    full_out = []
    for c in range(NCORES):
        pooled, wsum = decode_out(pl_all[c], dn_all[c], bcs)
        full_out.append(pooled / wsum[:, None])
    return best, np.concatenate(full_out, 0).astype(np.float32)


if __name__ == "__main__":
    rng = np.random.default_rng(0)
    ins = {
        "target_embedding": rng.standard_normal((B, D), dtype=np.float32),
        "hist_embeddings": rng.standard_normal((B, T, D), dtype=np.float32),
        "W_kernel": (rng.standard_normal((D, D), dtype=np.float32) / np.sqrt(D)),
        "W_bias": np.zeros(D, np.float32),
        "q_kernel": (rng.standard_normal((D, 1), dtype=np.float32) / np.sqrt(D)),
        "q_bias": np.zeros(1, np.float32),
    }
    out = kernel(**ins)
    print("out", out.shape, out.dtype)


# revision 30
# speedup vs baseline: 1.0053x; 1.0053x over previous
"""AttentionPooling Trainium2 kernel (v3: contiguous load + diagonal pooling).

Math (per batch row b):
    x   = target[b] + hist[b]              # [T, D]
    h   = relu(x @ W + Wb)                 # [T, D]
    lg  = h @ q (+ q_bias, softmax-invariant -> ignored)
    s   = softmax(lg)                      # over T
    out = sum_t s_t * hist[b, t]           # [D]

Device strategy (pure data parallel over batch across 8 cores).  Strided
HBM reads run at ~half bandwidth on real TRN2, so hist is loaded with a
single fully CONTIGUOUS fp32->bf16 cast DMA per 64-batch iteration into
the natural layout [p=(b,th), (tl,d)] (th = t//100, tl = t%100):
  - PE transposes the 100 [128,128] d-blocks -> xT [d, (tl, p)]; the
    PSUM->SBUF copy fuses the broadcast target add (packed APs keep the
    DVE 2x bf16 mode).
  - Main matmul: H^T = W^T @ xT (bf16, W stationary), relu+bias on ACT
    over 1024-col PSUM chunks.
  - q-matmul per tl-chunk: stationary = hh block [e,128], moving = q
    -> logits land NATURALLY as [p, tl] columns of one [128,100] PSUM
    tile; a single exp (ACT) with accum_out yields w AND the softmax
    denominators in one instruction.
  - Pooling: per tl one matmul, stationary = wdiag [128, 64] (w values
    scattered on the 2-diagonal (p, p//2), built by one DVE multiply
    with a constant 0/1 mask), moving = the natural hist block
    [128, 128] -> PSUM-accumulated [b, d] over all 100 tl.
  - Final normalize (divide by sum_t w) on host.
"""

import sys

sys.path.insert(0, "/opt/trn_rl_repo")

import numpy as np

import concourse.bacc as bacc
import concourse.bass as bass
import concourse.mybir as mybir
import concourse.tile as tile
from concourse import masks
from concourse.bass_utils import run_bass_kernel_spmd

F32 = mybir.dt.float32
BF16 = mybir.dt.bfloat16
AF = mybir.ActivationFunctionType

NCORES = 8
B, T, D = 16384, 200, 128
BC = B // NCORES          # 2048 batch rows per core
TL = 100                  # tl positions per partition (t = th*100 + tl)
B_IT = 64                 # batch rows per outer iteration
NC_IT = B_IT * T * D      # elements per iteration


def build(nc, b_core=BC, dbg=False):
    nit = b_core // B_IT
    hist = nc.dram_tensor("hist", [b_core, T, D], F32, kind="ExternalInput")
    tgt = nc.dram_tensor("target", [b_core, D], F32, kind="ExternalInput")
    w_in = nc.dram_tensor("W", [D, D], F32, kind="ExternalInput")
    wb_in = nc.dram_tensor("Wb", [D], F32, kind="ExternalInput")
    q_in = nc.dram_tensor("q", [D, 1], F32, kind="ExternalInput")
    out_pl = nc.dram_tensor("out_pl", [nit, B_IT, D], F32, kind="ExternalOutput")
    out_dn = nc.dram_tensor("out_dn", [nit, 128, 2], F32, kind="ExternalOutput")
    if dbg:
        dbg_nt = nc.dram_tensor("dbg_nt", [128, TL * D], F32, kind="ExternalOutput")
        dbg_ht = nc.dram_tensor("dbg_ht", [128, B_IT * T], F32, kind="ExternalOutput")
        dbg_hh = nc.dram_tensor("dbg_hh", [128, B_IT * T], F32, kind="ExternalOutput")
        dbg_w = nc.dram_tensor("dbg_w", [128, TL], F32, kind="ExternalOutput")
        dbg_wd = nc.dram_tensor("dbg_wd", [128, TL * B_IT], F32, kind="ExternalOutput")

    from contextlib import ExitStack
    with tile.TileContext(nc) as tc, ExitStack() as es:
        consts = es.enter_context(tc.tile_pool(name="consts", bufs=1))
        nt_pool = es.enter_context(tc.tile_pool(name="nt", bufs=CFG["nt"]))
        ht_pool = es.enter_context(tc.tile_pool(name="ht", bufs=CFG["ht"]))
        h_pool = es.enter_context(tc.tile_pool(name="h", bufs=CFG["hh"]))
        w_pool = es.enter_context(tc.tile_pool(name="w", bufs=CFG["wb"]))
        out_pool = es.enter_context(tc.tile_pool(name="out", bufs=CFG["outt"]))
        ps_tp = es.enter_context(tc.tile_pool(name="ps_tp", bufs=CFG["tp"], space="PSUM"))
        ps_mm = es.enter_context(tc.tile_pool(name="ps_mm", bufs=CFG["mm"], space="PSUM"))
        ps_q = es.enter_context(tc.tile_pool(name="ps_q", bufs=CFG["q"], space="PSUM"))
        ps_pool = es.enter_context(tc.tile_pool(name="ps_pool", bufs=CFG["pool"], space="PSUM"))

        # ---- constants ----
        ident = consts.tile([128, 128], BF16)
        masks.make_identity(nc, ident[:, :])

        w_f32 = consts.tile([D, D], F32)
        nc.sync.dma_start(out=w_f32, in_=w_in.ap())
        w_bf = consts.tile([D, D], BF16)
        nc.vector.tensor_copy(out=w_bf, in_=w_f32)

        wbias = consts.tile([D, 1], F32)
        nc.sync.dma_start(out=wbias, in_=wb_in.ap()[:, None])

        q_f32 = consts.tile([D, 1], F32)
        nc.sync.dma_start(out=q_f32, in_=q_in.ap())
        q_bf = consts.tile([D, 1], BF16)
        nc.vector.tensor_copy(out=q_bf, in_=q_f32)

        # 2-diagonal mask: I2[p, b] = 1 if p // 2 == b else 0  [128, 64] bf16
        # built from the identity: I2[p, b] = ident[p, 2b] + ident[p, 2b+1]
        i2 = consts.tile([128, B_IT], BF16)
        idv = ident.rearrange("p (b u) -> p b u", u=2)
        nc.vector.tensor_add(i2, idv[:, :, 0], idv[:, :, 1])

        # targetT [d, b_core] bf16
        tgtT = consts.tile([D, b_core], BF16)
        for k in range((b_core + 127) // 128):
            bn = min(128, b_core - k * 128)
            t_f32 = w_pool.tile([128, D], F32, tag="tsetup")
            nc.sync.dma_start(out=t_f32[0:bn], in_=tgt.ap()[k * 128:k * 128 + bn, :])
            t_bf = w_pool.tile([128, D], BF16, tag="tsetup_bf")
            nc.vector.tensor_copy(out=t_bf[0:bn], in_=t_f32[0:bn])
            tpp = ps_tp.tile([128, 1024], BF16, tag="tp")
            nc.tensor.transpose(tpp[:, 0:bn], t_bf[0:bn], ident[0:bn, 0:bn])
            nc.vector.tensor_copy(out=tgtT[:, k * 128:k * 128 + bn], in_=tpp[:, 0:bn])

        # ---- main loop ----
        # Pooling for iteration it is emitted during iteration it+1 (after
        # the transposes) so the PE never stalls on the exp -> wdiag chain.
        def emit_pool(prev):
            it_p, nt_p, wd_p = prev
            pl = ps_pool.tile([B_IT, D], F32)
            for tl in range(TL):
                nc.tensor.matmul(pl, wd_p[:, tl * B_IT:(tl + 1) * B_IT],
                                 nt_p[:, tl * D:tl * D + D],
                                 start=tl == 0, stop=tl == TL - 1)
            outt = out_pool.tile([B_IT, D], F32, tag="outt")
            nc.vector.tensor_copy(out=outt, in_=pl)
            nc.sync.dma_start(out=out_pl.ap()[it_p], in_=outt)

        prev = None
        for it in range(nit):
            b0 = it * B_IT

            # natural tile: partition p=(b,th), free (tl, d); one contiguous
            # cast DMA for the whole 64-batch slice
            nt = nt_pool.tile([128, TL * D], BF16, tag="nt")
            src = hist.ap()[b0:b0 + B_IT]
            nc.gpsimd.dma_start(
                out=nt,
                in_=bass.AP(tensor=src.tensor, offset=src.offset,
                            ap=[[TL * D, 128], [1, TL * D]]),
            )

            # target expanded 2x: tgx2[d, p] = tgtT[d, b0 + p//2] -- i.e.
            # column index IS p = 2b+th, so the broadcast AP below has a
            # fully packed innermost dim (DVE 2x mode)
            tgx2 = w_pool.tile([128, B_IT * 2], BF16, tag="tgx2")
            sl = tgtT[:, b0:b0 + B_IT]
            nc.vector.tensor_copy(
                out=tgx2,
                in_=bass.AP(tensor=sl.tensor, offset=sl.offset,
                            ap=[sl.ap[0], sl.ap[1], [0, 2]]),
            )

            # transposes -> xT [d, (tl, p)] with fused target add
            ht = ht_pool.tile([128, B_IT * T], BF16, tag="ht")
            NTG = CFG["ntg"]          # transposes per PSUM group (8 -> 1 bank)
            for g in range((TL + NTG - 1) // NTG) if "tp" not in SKIP else []:
                t0 = NTG * g
                ng = min(NTG, TL - t0)
                tp = ps_tp.tile([128, NTG * 128], BF16, tag="tp")
                for u in range(ng):
                    nc.tensor.transpose(
                        tp[:, 128 * u:128 * u + 128],
                        nt[:, (t0 + u) * D:(t0 + u) * D + D], ident)
                nc.vector.tensor_add(
                    ht.rearrange("d (t p) -> d t p",
                                 p=128)[:, t0:t0 + ng, :],
                    tp.rearrange("d (t p) -> d t p", p=128)[:, 0:ng, :],
                    bass.AP(tensor=tgx2.tensor, offset=tgx2.offset,
                            ap=[tgx2.ap[0], [0, ng], [1, 128]]),
                )

            if prev is not None and "pool" not in SKIP:
                emit_pool(prev)

            # H^T = relu(W^T xT + bias)  [e, (tl, p)]
            hh = h_pool.tile([128, B_IT * T], BF16, tag="hh")
            nmm = (B_IT * T) // 1024
            for k in range(nmm + 1) if "mm" not in SKIP else []:
                w_cols = 1024 if k < nmm else (B_IT * T) % 1024
                if w_cols == 0:
                    continue
                mm = ps_mm.tile([128, 1024], F32)
                for h in range((w_cols + 511) // 512):
                    c = 1024 * k + 512 * h
                    cw = min(512, w_cols - 512 * h)
                    nc.tensor.matmul(mm[:, 512 * h:512 * h + cw], w_bf,
                                     ht[:, c:c + cw], start=True, stop=True)
                nc.scalar.activation(hh[:, 1024 * k:1024 * k + w_cols],
                                     mm[:, 0:w_cols], AF.Relu, bias=wbias)

            # q-matmuls: logits land naturally [p, tl]
            qn = ps_q.tile([128, TL], F32)
            for tl in range(TL) if "q" not in SKIP else []:
                nc.tensor.matmul(qn[:, tl:tl + 1],
                                 hh[:, tl * 128:tl * 128 + 128], q_bf,
                                 start=True, stop=True)

            # exp in 2 chunks (pipelines with qn); accum gives denominators
            wnat = w_pool.tile([128, TL], BF16, tag="wnat")
            dn_sb = out_pool.tile([128, 2], F32, tag="dn")
            if "q" not in SKIP:
                for c in range(2):
                    nc.scalar.activation(wnat[:, 50 * c:50 * c + 50],
                                         qn[:, 50 * c:50 * c + 50], AF.Exp,
                                         accum_out=dn_sb[:, c:c + 1])
                nc.sync.dma_start(out=out_dn.ap()[it], in_=dn_sb)

            # wdiag build in 4 chunks: wdiag[p, (tl, b)] = I2[p, b] * wnat[p, tl]
            wdiag = w_pool.tile([128, TL * B_IT], BF16, tag="wdiag")
            wdv = wdiag.rearrange("p (t b) -> p t b", b=B_IT)
            if "pool" not in SKIP and "q" not in SKIP:
                for c in range(4):
                    wn = wnat[:, 25 * c:25 * c + 25]
                    nc.vector.tensor_mul(
                        wdv[:, 25 * c:25 * c + 25, :],
                        bass.AP(tensor=i2.tensor, offset=i2.offset,
                                ap=[i2.ap[0], [0, 25], i2.ap[1]]),
                        bass.AP(tensor=wn.tensor, offset=wn.offset,
                                ap=[wn.ap[0], wn.ap[1], [0, B_IT]]),
                    )
            prev = (it, nt, wdiag)

            if dbg and it == 0:
                nc.gpsimd.dma_start(out=dbg_nt.ap(), in_=nt)
                nc.gpsimd.dma_start(out=dbg_ht.ap(), in_=ht)
                nc.gpsimd.dma_start(out=dbg_hh.ap(), in_=hh)
                nc.gpsimd.dma_start(out=dbg_w.ap(), in_=wnat)
                nc.gpsimd.dma_start(out=dbg_wd.ap(), in_=wdiag)

        if prev is not None and "pool" not in SKIP:
            emit_pool(prev)

    return out_pl


def decode_out(pl, dn, b_core=BC):
    """out_pl [nit, B_IT, D], out_dn [nit, 128, 2] -> pooled, wsum per b."""
    nit = b_core // B_IT
    pooled = pl.reshape(b_core, D)
    d = dn.reshape(nit, B_IT, 2, 2).astype(np.float64)
    wsum = d.sum(axis=(2, 3)).reshape(b_core).astype(np.float32)
    return pooled, wsum


_cache = {}
LAST_RESULT = None
SKIP = set()
CFG = dict(nt=2, tp=2, mm=2, q=1, pool=1, ht=2, hh=1, outt=2, wb=3, ntg=8)


def _get_program(b_core):
    key = (b_core, tuple(sorted(SKIP)), tuple(sorted(CFG.items())))
    if key not in _cache:
        nc = bacc.Bacc("TRN2", target_bir_lowering=False, debug=False,
                       num_devices=NCORES)
        build(nc, b_core)
        nc.compile()
        _cache[key] = nc
    return _cache[key]


def kernel(**inputs):
    hist = np.ascontiguousarray(np.asarray(inputs["hist_embeddings"], np.float32))
    tgt = np.ascontiguousarray(np.asarray(inputs["target_embedding"], np.float32))
    W = np.ascontiguousarray(np.asarray(inputs["W_kernel"], np.float32))
    Wb = np.ascontiguousarray(np.asarray(inputs["W_bias"], np.float32))
    q = np.ascontiguousarray(np.asarray(inputs["q_kernel"], np.float32))
    # q_bias shifts every logit equally -> softmax-invariant -> ignored.

    nc = _get_program(BC)
    in_maps = []
    for c in range(NCORES):
        sl = slice(c * BC, (c + 1) * BC)
        in_maps.append({
            "hist": hist[sl], "target": tgt[sl],
            "W": W, "Wb": Wb, "q": q,
        })
    res = run_bass_kernel_spmd(nc, in_maps, core_ids=list(range(NCORES)))
    global LAST_RESULT
    LAST_RESULT = res
    outs = []
    for c in range(NCORES):
        pooled, wsum = decode_out(res.results[c]["out_pl"],
                                  res.results[c]["out_dn"])
        outs.append(pooled / wsum[:, None])
    return np.concatenate(outs, axis=0).astype(np.float32)


def timed_run(inputs, iters=5, bcs=BC):
    """Device-resident repeated execution; returns (best_seconds, outputs)."""
    import time
    import jax
    from jax.sharding import Mesh, PartitionSpec
    from jax.experimental.shard_map import shard_map
    import concourse.mybir as mybir_
    from concourse.bass2jax import (install_neuronx_cc_hook, _bass_exec_p,
                                    partition_id_tensor)

    hist = np.ascontiguousarray(np.asarray(inputs["hist_embeddings"], np.float32))
    tgt = np.ascontiguousarray(np.asarray(inputs["target_embedding"], np.float32))
    W = np.ascontiguousarray(np.asarray(inputs["W_kernel"], np.float32))
    Wb = np.ascontiguousarray(np.asarray(inputs["W_bias"], np.float32))
    q = np.ascontiguousarray(np.asarray(inputs["q_kernel"], np.float32))
    hist = hist[:NCORES * bcs].reshape(NCORES * bcs, T, D)
    tgt = tgt[:NCORES * bcs]
    nc = _get_program(bcs)
    install_neuronx_cc_hook()

    pid_name = nc.partition_id_tensor.name if nc.partition_id_tensor else None
    in_names, out_names, out_avals, zero_outs = [], [], [], []
    for alloc in nc.m.functions[0].allocations:
        if not isinstance(alloc, mybir_.MemoryLocationSet):
            continue
        name = alloc.memorylocations[0].name
        if alloc.kind == "ExternalInput":
            if name != pid_name:
                in_names.append(name)
        elif alloc.kind == "ExternalOutput":
            shape = tuple(alloc.tensor_shape)
            dtype = mybir_.dt.np(alloc.dtype)
            out_names.append(name)
            out_avals.append(jax.core.ShapedArray(shape, dtype))
            zero_outs.append(np.zeros(shape, dtype))
    all_names = in_names + out_names
    if pid_name is not None:
        all_names = all_names + [pid_name]

    import os
    chain = int(os.environ.get("KERNEL_CHAIN", "1"))

    def _body(*args):
        nin_ = len(in_names)
        ins_ = list(args[:nin_])
        outs = list(args[nin_:])
        for _ in range(chain):
            operands = ins_ + outs
            if pid_name is not None:
                operands = operands + [partition_id_tensor()]
            outs = list(_bass_exec_p.bind(
                *operands, out_avals=tuple(out_avals),
                in_names=tuple(all_names), out_names=tuple(out_names),
                lowering_input_output_aliases=(),
                sim_require_finite=True, sim_require_nnan=True, nc=nc))
        return tuple(outs)

    devices = jax.devices()[:NCORES]
    mesh = Mesh(np.array(devices), ("core",))
    nin = len(in_names) + len(out_names)
    fn = jax.jit(shard_map(_body, mesh=mesh,
                           in_specs=(PartitionSpec("core"),) * nin,
                           out_specs=(PartitionSpec("core"),) * len(out_names),
                           check_rep=False))
    full = {"hist": hist, "target": tgt,
            "W": np.concatenate([W] * NCORES, 0),
            "Wb": np.concatenate([Wb] * NCORES, 0),
            "q": np.concatenate([q] * NCORES, 0)}
    args = [full[n] for n in in_names] + [
        np.concatenate([z] * NCORES, 0) for z in zero_outs]
    sh = jax.sharding.NamedSharding(mesh, PartitionSpec("core"))
    dargs = [jax.device_put(a, sh) for a in args]
    res = fn(*dargs)
    jax.block_until_ready(res)
    import os
    pipeline = int(os.environ.get("KERNEL_PIPE", "1"))
    nin_ = len(in_names)
    best = float("inf")
    for _ in range(iters):
        t0 = time.perf_counter()
        r = tuple(dargs[nin_:])
        for _k in range(pipeline):
            r = fn(*dargs[:nin_], *r)
        jax.block_until_ready(r)
        best = min(best, time.perf_counter() - t0)
        res = r
    outs = [np.asarray(r) for r in res]
    pl_all = np.split(outs[out_names.index("out_pl")], NCORES, axis=0)
    dn_all = np.split(outs[out_names.index("out_dn")], NCORES, axis=0)
    full_out = []
    for c in range(NCORES):
        pooled, wsum = decode_out(pl_all[c], dn_all[c], bcs)
        full_out.append(pooled / wsum[:, None])
    return best, np.concatenate(full_out, 0).astype(np.float32)


if __name__ == "__main__":
    rng = np.random.default_rng(0)
    ins = {
        "target_embedding": rng.standard_normal((B, D), dtype=np.float32),
        "hist_embeddings": rng.standard_normal((B, T, D), dtype=np.float32),
        "W_kernel": (rng.standard_normal((D, D), dtype=np.float32) / np.sqrt(D)),
        "W_bias": np.zeros(D, np.float32),
        "q_kernel": (rng.standard_normal((D, 1), dtype=np.float32) / np.sqrt(D)),
        "q_bias": np.zeros(1, np.float32),
    }
    out = kernel(**ins)
    print("out", out.shape, out.dtype)


# revision 33
# speedup vs baseline: 1.0726x; 1.0670x over previous
"""AttentionPooling Trainium2 kernel (v3: contiguous load + diagonal pooling).

Math (per batch row b):
    x   = target[b] + hist[b]              # [T, D]
    h   = relu(x @ W + Wb)                 # [T, D]
    lg  = h @ q (+ q_bias, softmax-invariant -> ignored)
    s   = softmax(lg)                      # over T
    out = sum_t s_t * hist[b, t]           # [D]

Device strategy (pure data parallel over batch across 8 cores).  Strided
HBM reads run at ~half bandwidth on real TRN2, so hist is loaded with a
single fully CONTIGUOUS fp32->bf16 cast DMA per 64-batch iteration into
the natural layout [p=(b,th), (tl,d)] (th = t//100, tl = t%100):
  - PE transposes the 100 [128,128] d-blocks -> xT [d, (tl, p)]; the
    PSUM->SBUF copy fuses the broadcast target add (packed APs keep the
    DVE 2x bf16 mode).
  - Main matmul: H^T = W^T @ xT (bf16, W stationary), relu+bias on ACT
    over 1024-col PSUM chunks.
  - q-matmul per tl-chunk: stationary = hh block [e,128], moving = q
    -> logits land NATURALLY as [p, tl] columns of one [128,100] PSUM
    tile; a single exp (ACT) with accum_out yields w AND the softmax
    denominators in one instruction.
  - Pooling: per tl one matmul, stationary = wdiag [128, 64] (w values
    scattered on the 2-diagonal (p, p//2), built by one DVE multiply
    with a constant 0/1 mask), moving = the natural hist block
    [128, 128] -> PSUM-accumulated [b, d] over all 100 tl.
  - Final normalize (divide by sum_t w) on host.
"""

import sys

sys.path.insert(0, "/opt/trn_rl_repo")

import numpy as np

import concourse.bacc as bacc
import concourse.bass as bass
import concourse.mybir as mybir
import concourse.tile as tile
from concourse import masks
from concourse.bass_utils import run_bass_kernel_spmd

F32 = mybir.dt.float32
BF16 = mybir.dt.bfloat16
AF = mybir.ActivationFunctionType

NCORES = 8
B, T, D = 16384, 200, 128
BC = B // NCORES          # 2048 batch rows per core
TL = 100                  # tl positions per partition (t = th*100 + tl)
B_IT = 64                 # batch rows per outer iteration
NC_IT = B_IT * T * D      # elements per iteration


def build(nc, b_core=BC, dbg=False):
    nit = b_core // B_IT
    hist = nc.dram_tensor("hist", [b_core, T, D], F32, kind="ExternalInput")
    tgt = nc.dram_tensor("target", [b_core, D], F32, kind="ExternalInput")
    w_in = nc.dram_tensor("W", [D, D], F32, kind="ExternalInput")
    wb_in = nc.dram_tensor("Wb", [D], F32, kind="ExternalInput")
    q_in = nc.dram_tensor("q", [D, 1], F32, kind="ExternalInput")
    out_pl = nc.dram_tensor("out_pl", [nit, B_IT, D], F32, kind="ExternalOutput")
    out_dn = nc.dram_tensor("out_dn", [nit, 128, 2], F32, kind="ExternalOutput")
    if dbg:
        dbg_nt = nc.dram_tensor("dbg_nt", [128, TL * D], F32, kind="ExternalOutput")
        dbg_ht = nc.dram_tensor("dbg_ht", [128, B_IT * T], F32, kind="ExternalOutput")
        dbg_hh = nc.dram_tensor("dbg_hh", [128, B_IT * T], F32, kind="ExternalOutput")
        dbg_w = nc.dram_tensor("dbg_w", [128, TL], F32, kind="ExternalOutput")
        dbg_wd = nc.dram_tensor("dbg_wd", [128, TL * B_IT], F32, kind="ExternalOutput")

    from contextlib import ExitStack
    with tile.TileContext(nc) as tc, ExitStack() as es:
        consts = es.enter_context(tc.tile_pool(name="consts", bufs=1))
        nt_pool = es.enter_context(tc.tile_pool(name="nt", bufs=CFG["nt"]))
        ht_pool = es.enter_context(tc.tile_pool(name="ht", bufs=CFG["ht"]))
        h_pool = es.enter_context(tc.tile_pool(name="h", bufs=CFG["hh"]))
        w_pool = es.enter_context(tc.tile_pool(name="w", bufs=CFG["wb"]))
        out_pool = es.enter_context(tc.tile_pool(name="out", bufs=CFG["outt"]))
        ps_tp = es.enter_context(tc.tile_pool(name="ps_tp", bufs=CFG["tp"], space="PSUM"))
        ps_mm = es.enter_context(tc.tile_pool(name="ps_mm", bufs=CFG["mm"], space="PSUM"))
        ps_q = es.enter_context(tc.tile_pool(name="ps_q", bufs=CFG["q"], space="PSUM"))
        ps_pool = es.enter_context(tc.tile_pool(name="ps_pool", bufs=CFG["pool"], space="PSUM"))

        # ---- constants ----
        ident = consts.tile([128, 128], BF16)
        masks.make_identity(nc, ident[:, :])

        w_f32 = consts.tile([D, D], F32)
        nc.sync.dma_start(out=w_f32, in_=w_in.ap())
        w_bf = consts.tile([D, D], BF16)
        nc.vector.tensor_copy(out=w_bf, in_=w_f32)

        wbias = consts.tile([D, 1], F32)
        nc.sync.dma_start(out=wbias, in_=wb_in.ap()[:, None])

        q_f32 = consts.tile([D, 1], F32)
        nc.sync.dma_start(out=q_f32, in_=q_in.ap())
        q_bf = consts.tile([D, 1], BF16)
        nc.vector.tensor_copy(out=q_bf, in_=q_f32)

        # 2-diagonal mask: I2[p, b] = 1 if p // 2 == b else 0  [128, 64] bf16
        # built from the identity: I2[p, b] = ident[p, 2b] + ident[p, 2b+1]
        i2 = consts.tile([128, B_IT], BF16)
        idv = ident.rearrange("p (b u) -> p b u", u=2)
        nc.vector.tensor_add(i2, idv[:, :, 0], idv[:, :, 1])

        # targetT [d, b_core] bf16
        tgtT = consts.tile([D, b_core], BF16)
        for k in range((b_core + 127) // 128):
            bn = min(128, b_core - k * 128)
            t_f32 = w_pool.tile([128, D], F32, tag="tsetup")
            nc.sync.dma_start(out=t_f32[0:bn], in_=tgt.ap()[k * 128:k * 128 + bn, :])
            t_bf = w_pool.tile([128, D], BF16, tag="tsetup_bf")
            nc.vector.tensor_copy(out=t_bf[0:bn], in_=t_f32[0:bn])
            tpp = ps_tp.tile([128, 1024], BF16, tag="tp")
            nc.tensor.transpose(tpp[:, 0:bn], t_bf[0:bn], ident[0:bn, 0:bn])
            nc.vector.tensor_copy(out=tgtT[:, k * 128:k * 128 + bn], in_=tpp[:, 0:bn])

        # ---- main loop ----
        # Pooling for iteration it is emitted during iteration it+1 (after
        # the transposes) so the PE never stalls on the exp -> wdiag chain.
        def emit_pool(prev):
            it_p, nt_p, wd_p = prev
            pl = ps_pool.tile([B_IT, D], F32)
            for tl in range(TL):
                nc.tensor.matmul(pl, wd_p[:, tl * B_IT:(tl + 1) * B_IT],
                                 nt_p[:, tl * D:tl * D + D],
                                 start=tl == 0, stop=tl == TL - 1)
            outt = out_pool.tile([B_IT, D], F32, tag="outt")
            nc.vector.tensor_copy(out=outt, in_=pl)
            nc.sync.dma_start(out=out_pl.ap()[it_p], in_=outt)

        prev = None
        for it in range(nit):
            b0 = it * B_IT

            # natural tile: partition p=(b,th), free (tl, d); one contiguous
            # cast DMA for the whole 64-batch slice
            nt = nt_pool.tile([128, TL * D], BF16, tag="nt")
            src = hist.ap()[b0:b0 + B_IT]
            nc.gpsimd.dma_start(
                out=nt,
                in_=bass.AP(tensor=src.tensor, offset=src.offset,
                            ap=[[TL * D, 128], [1, TL * D]]),
            )

            # target expanded 2x: tgx2[d, p] = tgtT[d, b0 + p//2] -- i.e.
            # column index IS p = 2b+th, so the broadcast AP below has a
            # fully packed innermost dim (DVE 2x mode)
            tgx2 = w_pool.tile([128, B_IT * 2], BF16, tag="tgx2")
            sl = tgtT[:, b0:b0 + B_IT]
            nc.vector.tensor_copy(
                out=tgx2,
                in_=bass.AP(tensor=sl.tensor, offset=sl.offset,
                            ap=[sl.ap[0], sl.ap[1], [0, 2]]),
            )

            # transposes -> xT [d, (tl, p)] with fused target add
            ht = ht_pool.tile([128, B_IT * T], BF16, tag="ht")
            NTG = CFG["ntg"]          # transposes per PSUM group (8 -> 1 bank)
            for g in range((TL + NTG - 1) // NTG) if "tp" not in SKIP else []:
                t0 = NTG * g
                ng = min(NTG, TL - t0)
                tp = ps_tp.tile([128, NTG * 128], BF16, tag="tp")
                for u in range(ng):
                    nc.tensor.transpose(
                        tp[:, 128 * u:128 * u + 128],
                        nt[:, (t0 + u) * D:(t0 + u) * D + D], ident)
                nc.vector.tensor_add(
                    ht.rearrange("d (t p) -> d t p",
                                 p=128)[:, t0:t0 + ng, :],
                    tp.rearrange("d (t p) -> d t p", p=128)[:, 0:ng, :],
                    bass.AP(tensor=tgx2.tensor, offset=tgx2.offset,
                            ap=[tgx2.ap[0], [0, ng], [1, 128]]),
                )

            if prev is not None and "pool" not in SKIP:
                emit_pool(prev)

            # H^T = relu(W^T xT + bias)  [e, (tl, p)]
            hh = h_pool.tile([128, B_IT * T], BF16, tag="hh")
            nmm = (B_IT * T) // 1024
            for k in range(nmm + 1) if "mm" not in SKIP else []:
                w_cols = 1024 if k < nmm else (B_IT * T) % 1024
                if w_cols == 0:
                    continue
                mm = ps_mm.tile([128, 1024], F32)
                for h in range((w_cols + 511) // 512):
                    c = 1024 * k + 512 * h
                    cw = min(512, w_cols - 512 * h)
                    nc.tensor.matmul(mm[:, 512 * h:512 * h + cw], w_bf,
                                     ht[:, c:c + cw], start=True, stop=True)
                nc.scalar.activation(hh[:, 1024 * k:1024 * k + w_cols],
                                     mm[:, 0:w_cols], AF.Relu, bias=wbias)

            # q-matmuls: logits land naturally [p, tl]
            qn = ps_q.tile([128, TL], F32)
            for tl in range(TL) if "q" not in SKIP else []:
                nc.tensor.matmul(qn[:, tl:tl + 1],
                                 hh[:, tl * 128:tl * 128 + 128], q_bf,
                                 start=True, stop=True)

            # exp in 2 chunks (pipelines with qn); accum gives denominators
            wnat = w_pool.tile([128, TL], BF16, tag="wnat")
            dn_sb = out_pool.tile([128, 2], F32, tag="dn")
            if "q" not in SKIP:
                for c in range(2):
                    nc.scalar.activation(wnat[:, 50 * c:50 * c + 50],
                                         qn[:, 50 * c:50 * c + 50], AF.Exp,
                                         accum_out=dn_sb[:, c:c + 1])
                nc.sync.dma_start(out=out_dn.ap()[it], in_=dn_sb)

            # wdiag build in 4 chunks: wdiag[p, (tl, b)] = I2[p, b] * wnat[p, tl]
            wdiag = w_pool.tile([128, TL * B_IT], BF16, tag="wdiag")
            wdv = wdiag.rearrange("p (t b) -> p t b", b=B_IT)
            if "pool" not in SKIP and "q" not in SKIP:
                for c in range(4):
                    wn = wnat[:, 25 * c:25 * c + 25]
                    nc.vector.tensor_mul(
                        wdv[:, 25 * c:25 * c + 25, :],
                        bass.AP(tensor=i2.tensor, offset=i2.offset,
                                ap=[i2.ap[0], [0, 25], i2.ap[1]]),
                        bass.AP(tensor=wn.tensor, offset=wn.offset,
                                ap=[wn.ap[0], wn.ap[1], [0, B_IT]]),
                    )
            prev = (it, nt, wdiag)

            if dbg and it == 0:
                nc.gpsimd.dma_start(out=dbg_nt.ap(), in_=nt)
                nc.gpsimd.dma_start(out=dbg_ht.ap(), in_=ht)
                nc.gpsimd.dma_start(out=dbg_hh.ap(), in_=hh)
                nc.gpsimd.dma_start(out=dbg_w.ap(), in_=wnat)
                nc.gpsimd.dma_start(out=dbg_wd.ap(), in_=wdiag)

        if prev is not None and "pool" not in SKIP:
            emit_pool(prev)

    return out_pl


def decode_out(pl, dn, b_core=BC):
    """out_pl [nit, B_IT, D], out_dn [nit, 128, 2] -> pooled, wsum per b."""
    nit = b_core // B_IT
    pooled = pl.reshape(b_core, D)
    d = dn.reshape(nit, B_IT, 2, 2).astype(np.float64)
    wsum = d.sum(axis=(2, 3)).reshape(b_core).astype(np.float32)
    return pooled, wsum


_cache = {}
LAST_RESULT = None
SKIP = set()
CFG = dict(nt=3, tp=2, mm=2, q=1, pool=1, ht=2, hh=1, outt=2, wb=3, ntg=8)


def _get_program(b_core):
    key = (b_core, tuple(sorted(SKIP)), tuple(sorted(CFG.items())))
    if key not in _cache:
        nc = bacc.Bacc("TRN2", target_bir_lowering=False, debug=False,
                       num_devices=NCORES)
        build(nc, b_core)
        nc.compile()
        _cache[key] = nc
    return _cache[key]


def kernel(**inputs):
    hist = np.ascontiguousarray(np.asarray(inputs["hist_embeddings"], np.float32))
    tgt = np.ascontiguousarray(np.asarray(inputs["target_embedding"], np.float32))
    W = np.ascontiguousarray(np.asarray(inputs["W_kernel"], np.float32))
    Wb = np.ascontiguousarray(np.asarray(inputs["W_bias"], np.float32))
    q = np.ascontiguousarray(np.asarray(inputs["q_kernel"], np.float32))
    # q_bias shifts every logit equally -> softmax-invariant -> ignored.

    nc = _get_program(BC)
    in_maps = []
    for c in range(NCORES):
        sl = slice(c * BC, (c + 1) * BC)
        in_maps.append({
            "hist": hist[sl], "target": tgt[sl],
            "W": W, "Wb": Wb, "q": q,
        })
    res = run_bass_kernel_spmd(nc, in_maps, core_ids=list(range(NCORES)))
    global LAST_RESULT
    LAST_RESULT = res
    outs = []
    for c in range(NCORES):
        pooled, wsum = decode_out(res.results[c]["out_pl"],
                                  res.results[c]["out_dn"])
        outs.append(pooled / wsum[:, None])
    return np.concatenate(outs, axis=0).astype(np.float32)


def timed_run(inputs, iters=5, bcs=BC):
    """Device-resident repeated execution; returns (best_seconds, outputs)."""
    import time
    import jax
    from jax.sharding import Mesh, PartitionSpec
    from jax.experimental.shard_map import shard_map
    import concourse.mybir as mybir_
    from concourse.bass2jax import (install_neuronx_cc_hook, _bass_exec_p,
                                    partition_id_tensor)

    hist = np.ascontiguousarray(np.asarray(inputs["hist_embeddings"], np.float32))
    tgt = np.ascontiguousarray(np.asarray(inputs["target_embedding"], np.float32))
    W = np.ascontiguousarray(np.asarray(inputs["W_kernel"], np.float32))
    Wb = np.ascontiguousarray(np.asarray(inputs["W_bias"], np.float32))
    q = np.ascontiguousarray(np.asarray(inputs["q_kernel"], np.float32))
    hist = hist[:NCORES * bcs].reshape(NCORES * bcs, T, D)
    tgt = tgt[:NCORES * bcs]
    nc = _get_program(bcs)
    install_neuronx_cc_hook()

    pid_name = nc.partition_id_tensor.name if nc.partition_id_tensor else None
    in_names, out_names, out_avals, zero_outs = [], [], [], []
    for alloc in nc.m.functions[0].allocations:
        if not isinstance(alloc, mybir_.MemoryLocationSet):
            continue
        name = alloc.memorylocations[0].name
        if alloc.kind == "ExternalInput":
            if name != pid_name:
                in_names.append(name)
        elif alloc.kind == "ExternalOutput":
            shape = tuple(alloc.tensor_shape)
            dtype = mybir_.dt.np(alloc.dtype)
            out_names.append(name)
            out_avals.append(jax.core.ShapedArray(shape, dtype))
            zero_outs.append(np.zeros(shape, dtype))
    all_names = in_names + out_names
    if pid_name is not None:
        all_names = all_names + [pid_name]

    import os
    chain = int(os.environ.get("KERNEL_CHAIN", "1"))

    def _body(*args):
        nin_ = len(in_names)
        ins_ = list(args[:nin_])
        outs = list(args[nin_:])
        for _ in range(chain):
            operands = ins_ + outs
            if pid_name is not None:
                operands = operands + [partition_id_tensor()]
            outs = list(_bass_exec_p.bind(
                *operands, out_avals=tuple(out_avals),
                in_names=tuple(all_names), out_names=tuple(out_names),
                lowering_input_output_aliases=(),
                sim_require_finite=True, sim_require_nnan=True, nc=nc))
        return tuple(outs)

    devices = jax.devices()[:NCORES]
    mesh = Mesh(np.array(devices), ("core",))
    nin = len(in_names) + len(out_names)
    fn = jax.jit(shard_map(_body, mesh=mesh,
                           in_specs=(PartitionSpec("core"),) * nin,
                           out_specs=(PartitionSpec("core"),) * len(out_names),
                           check_rep=False))
    full = {"hist": hist, "target": tgt,
            "W": np.concatenate([W] * NCORES, 0),
            "Wb": np.concatenate([Wb] * NCORES, 0),
            "q": np.concatenate([q] * NCORES, 0)}
    args = [full[n] for n in in_names] + [
        np.concatenate([z] * NCORES, 0) for z in zero_outs]
    sh = jax.sharding.NamedSharding(mesh, PartitionSpec("core"))
    dargs = [jax.device_put(a, sh) for a in args]
    res = fn(*dargs)
    jax.block_until_ready(res)
    import os
    pipeline = int(os.environ.get("KERNEL_PIPE", "1"))
    nin_ = len(in_names)
    best = float("inf")
    for _ in range(iters):
        t0 = time.perf_counter()
        r = tuple(dargs[nin_:])
        for _k in range(pipeline):
            r = fn(*dargs[:nin_], *r)
        jax.block_until_ready(r)
        best = min(best, time.perf_counter() - t0)
        res = r
    outs = [np.asarray(r) for r in res]
    pl_all = np.split(outs[out_names.index("out_pl")], NCORES, axis=0)
    dn_all = np.split(outs[out_names.index("out_dn")], NCORES, axis=0)
    full_out = []
    for c in range(NCORES):
        pooled, wsum = decode_out(pl_all[c], dn_all[c], bcs)
        full_out.append(pooled / wsum[:, None])
    return best, np.concatenate(full_out, 0).astype(np.float32)


if __name__ == "__main__":
    rng = np.random.default_rng(0)
    ins = {
        "target_embedding": rng.standard_normal((B, D), dtype=np.float32),
        "hist_embeddings": rng.standard_normal((B, T, D), dtype=np.float32),
        "W_kernel": (rng.standard_normal((D, D), dtype=np.float32) / np.sqrt(D)),
        "W_bias": np.zeros(D, np.float32),
        "q_kernel": (rng.standard_normal((D, 1), dtype=np.float32) / np.sqrt(D)),
        "q_bias": np.zeros(1, np.float32),
    }
    out = kernel(**ins)
    print("out", out.shape, out.dtype)


# revision 35
# speedup vs baseline: 1.4132x; 1.3174x over previous
"""AttentionPooling Trainium2 kernel (v3: contiguous load + diagonal pooling).

Math (per batch row b):
    x   = target[b] + hist[b]              # [T, D]
    h   = relu(x @ W + Wb)                 # [T, D]
    lg  = h @ q (+ q_bias, softmax-invariant -> ignored)
    s   = softmax(lg)                      # over T
    out = sum_t s_t * hist[b, t]           # [D]

Device strategy (pure data parallel over batch across 8 cores).  Strided
HBM reads run at ~half bandwidth on real TRN2, so hist is loaded with a
single fully CONTIGUOUS fp32->bf16 cast DMA per 64-batch iteration into
the natural layout [p=(b,th), (tl,d)] (th = t//100, tl = t%100):
  - PE transposes the 100 [128,128] d-blocks -> xT [d, (tl, p)]; the
    PSUM->SBUF copy fuses the broadcast target add (packed APs keep the
    DVE 2x bf16 mode).
  - Main matmul: H^T = W^T @ xT (bf16, W stationary), relu+bias on ACT
    over 1024-col PSUM chunks.
  - q-matmul per tl-chunk: stationary = hh block [e,128], moving = q
    -> logits land NATURALLY as [p, tl] columns of one [128,100] PSUM
    tile; a single exp (ACT) with accum_out yields w AND the softmax
    denominators in one instruction.
  - Pooling: per tl one matmul, stationary = wdiag [128, 64] (w values
    scattered on the 2-diagonal (p, p//2), built by one DVE multiply
    with a constant 0/1 mask), moving = the natural hist block
    [128, 128] -> PSUM-accumulated [b, d] over all 100 tl.
  - Final normalize (divide by sum_t w) on host.
"""

import sys

sys.path.insert(0, "/opt/trn_rl_repo")

import numpy as np

import concourse.bacc as bacc
import concourse.bass as bass
import concourse.mybir as mybir
import concourse.tile as tile
from concourse import masks
from concourse.bass_utils import run_bass_kernel_spmd

F32 = mybir.dt.float32
BF16 = mybir.dt.bfloat16
AF = mybir.ActivationFunctionType

NCORES = 8
B, T, D = 16384, 200, 128
BC = B // NCORES          # 2048 batch rows per core
TL = 100                  # tl positions per partition (t = th*100 + tl)
B_IT = 64                 # batch rows per outer iteration
NC_IT = B_IT * T * D      # elements per iteration


def build(nc, b_core=BC, dbg=False):
    nit = b_core // B_IT
    hist = nc.dram_tensor("hist", [b_core, T, D], F32, kind="ExternalInput")
    tgt = nc.dram_tensor("target", [b_core, D], F32, kind="ExternalInput")
    w_in = nc.dram_tensor("W", [D, D], F32, kind="ExternalInput")
    wb_in = nc.dram_tensor("Wb", [D], F32, kind="ExternalInput")
    q_in = nc.dram_tensor("q", [D, 1], F32, kind="ExternalInput")
    out_pl = nc.dram_tensor("out_pl", [nit, B_IT, D], F32, kind="ExternalOutput")
    out_dn = nc.dram_tensor("out_dn", [nit, 128, 2], F32, kind="ExternalOutput")
    if dbg:
        dbg_nt = nc.dram_tensor("dbg_nt", [128, TL * D], F32, kind="ExternalOutput")
        dbg_ht = nc.dram_tensor("dbg_ht", [128, B_IT * T], F32, kind="ExternalOutput")
        dbg_hh = nc.dram_tensor("dbg_hh", [128, B_IT * T], F32, kind="ExternalOutput")
        dbg_w = nc.dram_tensor("dbg_w", [128, TL], F32, kind="ExternalOutput")
        dbg_wd = nc.dram_tensor("dbg_wd", [128, TL * B_IT], F32, kind="ExternalOutput")

    from contextlib import ExitStack
    with tile.TileContext(nc) as tc, ExitStack() as es:
        consts = es.enter_context(tc.tile_pool(name="consts", bufs=1))
        nt_pool = es.enter_context(tc.tile_pool(name="nt", bufs=CFG["nt"]))
        ht_pool = es.enter_context(tc.tile_pool(name="ht", bufs=CFG["ht"]))
        h_pool = es.enter_context(tc.tile_pool(name="h", bufs=CFG["hh"]))
        w_pool = es.enter_context(tc.tile_pool(name="w", bufs=CFG["wb"]))
        out_pool = es.enter_context(tc.tile_pool(name="out", bufs=CFG["outt"]))
        ps_tp = es.enter_context(tc.tile_pool(name="ps_tp", bufs=CFG["tp"], space="PSUM"))
        ps_mm = es.enter_context(tc.tile_pool(name="ps_mm", bufs=CFG["mm"], space="PSUM"))
        ps_q = es.enter_context(tc.tile_pool(name="ps_q", bufs=CFG["q"], space="PSUM"))
        ps_pool = es.enter_context(tc.tile_pool(name="ps_pool", bufs=CFG["pool"], space="PSUM"))

        # ---- constants ----
        ident = consts.tile([128, 128], BF16)
        masks.make_identity(nc, ident[:, :])

        w_f32 = consts.tile([D, D], F32)
        nc.sync.dma_start(out=w_f32, in_=w_in.ap())
        w_bf = consts.tile([D, D], BF16)
        nc.vector.tensor_copy(out=w_bf, in_=w_f32)

        wbias = consts.tile([D, 1], F32)
        nc.sync.dma_start(out=wbias, in_=wb_in.ap()[:, None])

        q_f32 = consts.tile([D, 1], F32)
        nc.sync.dma_start(out=q_f32, in_=q_in.ap())
        q_bf = consts.tile([D, 1], BF16)
        nc.vector.tensor_copy(out=q_bf, in_=q_f32)

        # 2-diagonal mask: I2[p, b] = 1 if p // 2 == b else 0  [128, 64] bf16
        # built from the identity: I2[p, b] = ident[p, 2b] + ident[p, 2b+1]
        i2 = consts.tile([128, B_IT], BF16)
        idv = ident.rearrange("p (b u) -> p b u", u=2)
        nc.vector.tensor_add(i2, idv[:, :, 0], idv[:, :, 1])

        # targetT [d, b_core] bf16
        tgtT = consts.tile([D, b_core], BF16)
        for k in range((b_core + 127) // 128):
            bn = min(128, b_core - k * 128)
            t_f32 = w_pool.tile([128, D], F32, tag="tsetup")
            nc.sync.dma_start(out=t_f32[0:bn], in_=tgt.ap()[k * 128:k * 128 + bn, :])
            t_bf = w_pool.tile([128, D], BF16, tag="tsetup_bf")
            nc.vector.tensor_copy(out=t_bf[0:bn], in_=t_f32[0:bn])
            tpp = ps_tp.tile([128, 1024], BF16, tag="tp")
            nc.tensor.transpose(tpp[:, 0:bn], t_bf[0:bn], ident[0:bn, 0:bn])
            nc.vector.tensor_copy(out=tgtT[:, k * 128:k * 128 + bn], in_=tpp[:, 0:bn])

        # ---- main loop ----
        # Pooling for iteration it is emitted during iteration it+1 (after
        # the transposes) so the PE never stalls on the exp -> wdiag chain.
        def emit_pool(prev):
            it_p, nt_p, wd_p = prev
            pl = ps_pool.tile([B_IT, D], F32)
            for tl in range(TL):
                nc.tensor.matmul(pl, wd_p[:, tl * B_IT:(tl + 1) * B_IT],
                                 nt_p[:, tl * D:tl * D + D],
                                 start=tl == 0, stop=tl == TL - 1)
            outt = out_pool.tile([B_IT, D], F32, tag="outt")
            nc.vector.tensor_copy(out=outt, in_=pl)
            nc.sync.dma_start(out=out_pl.ap()[it_p], in_=outt)

        prev = None
        for it in range(nit):
            b0 = it * B_IT

            # natural tile: partition p=(b,th), free (tl, d); one contiguous
            # cast DMA for the whole 64-batch slice
            nt = nt_pool.tile([128, TL * D], BF16, tag="nt")
            src = hist.ap()[b0:b0 + B_IT]
            nc.gpsimd.dma_start(
                out=nt,
                in_=bass.AP(tensor=src.tensor, offset=src.offset,
                            ap=[[TL * D, 128], [1, TL * D]]),
            )

            # target expanded 2x: tgx2[d, p] = tgtT[d, b0 + p//2] -- i.e.
            # column index IS p = 2b+th, so the broadcast AP below has a
            # fully packed innermost dim (DVE 2x mode)
            tgx2 = w_pool.tile([128, B_IT * 2], BF16, tag="tgx2")
            sl = tgtT[:, b0:b0 + B_IT]
            nc.vector.tensor_copy(
                out=tgx2,
                in_=bass.AP(tensor=sl.tensor, offset=sl.offset,
                            ap=[sl.ap[0], sl.ap[1], [0, 2]]),
            )

            # transposes -> xT [d, (tl, p)] with fused target add
            ht = ht_pool.tile([128, B_IT * T], BF16, tag="ht")
            NTG = CFG["ntg"]          # transposes per PSUM group (8 -> 1 bank)
            for g in range((TL + NTG - 1) // NTG) if "tp" not in SKIP else []:
                t0 = NTG * g
                ng = min(NTG, TL - t0)
                tp = ps_tp.tile([128, NTG * 128], BF16, tag="tp")
                for u in range(ng):
                    nc.tensor.transpose(
                        tp[:, 128 * u:128 * u + 128],
                        nt[:, (t0 + u) * D:(t0 + u) * D + D], ident)
                nc.vector.tensor_add(
                    ht.rearrange("d (t p) -> d t p",
                                 p=128)[:, t0:t0 + ng, :],
                    tp.rearrange("d (t p) -> d t p", p=128)[:, 0:ng, :],
                    bass.AP(tensor=tgx2.tensor, offset=tgx2.offset,
                            ap=[tgx2.ap[0], [0, ng], [1, 128]]),
                )

            if prev is not None and "pool" not in SKIP and not CFG.get("ilv"):
                emit_pool(prev)

            # H^T = relu(W^T xT + bias)  [e, (tl, p)]
            hh = h_pool.tile([128, B_IT * T], BF16, tag="hh")
            nmm = (B_IT * T) // 1024
            for k in range(nmm + 1) if "mm" not in SKIP else []:
                w_cols = 1024 if k < nmm else (B_IT * T) % 1024
                if w_cols == 0:
                    continue
                mm = ps_mm.tile([128, 1024], F32)
                for h in range((w_cols + 511) // 512):
                    c = 1024 * k + 512 * h
                    cw = min(512, w_cols - 512 * h)
                    nc.tensor.matmul(mm[:, 512 * h:512 * h + cw], w_bf,
                                     ht[:, c:c + cw], start=True, stop=True)
                nc.scalar.activation(hh[:, 1024 * k:1024 * k + w_cols],
                                     mm[:, 0:w_cols], AF.Relu, bias=wbias)

            # q-matmuls: logits land naturally [p, tl].  With CFG["ilv"],
            # the previous iteration's pool matmuls are interleaved so the
            # qn stationary loads hide under their 128-col streams.
            qn = ps_q.tile([128, TL], F32)
            do_ilv = (CFG.get("ilv") and prev is not None
                      and "pool" not in SKIP)
            if do_ilv:
                it_p, nt_p, wd_p = prev
                pl = ps_pool.tile([B_IT, D], F32)
            for tl in range(TL) if "q" not in SKIP else []:
                nc.tensor.matmul(qn[:, tl:tl + 1],
                                 hh[:, tl * 128:tl * 128 + 128], q_bf,
                                 start=True, stop=True)
                if do_ilv:
                    nc.tensor.matmul(pl, wd_p[:, tl * B_IT:(tl + 1) * B_IT],
                                     nt_p[:, tl * D:tl * D + D],
                                     start=tl == 0, stop=tl == TL - 1)
            if do_ilv:
                outt = out_pool.tile([B_IT, D], F32, tag="outt")
                nc.vector.tensor_copy(out=outt, in_=pl)
                nc.sync.dma_start(out=out_pl.ap()[it_p], in_=outt)

            # exp in 2 chunks (pipelines with qn); accum gives denominators
            wnat = w_pool.tile([128, TL], BF16, tag="wnat")
            dn_sb = out_pool.tile([128, 2], F32, tag="dn")
            if "q" not in SKIP:
                for c in range(2):
                    nc.scalar.activation(wnat[:, 50 * c:50 * c + 50],
                                         qn[:, 50 * c:50 * c + 50], AF.Exp,
                                         accum_out=dn_sb[:, c:c + 1])
                nc.sync.dma_start(out=out_dn.ap()[it], in_=dn_sb)

            # wdiag build in 4 chunks: wdiag[p, (tl, b)] = I2[p, b] * wnat[p, tl]
            wdiag = w_pool.tile([128, TL * B_IT], BF16, tag="wdiag")
            wdv = wdiag.rearrange("p (t b) -> p t b", b=B_IT)
            if "pool" not in SKIP and "q" not in SKIP:
                for c in range(4):
                    wn = wnat[:, 25 * c:25 * c + 25]
                    nc.vector.tensor_mul(
                        wdv[:, 25 * c:25 * c + 25, :],
                        bass.AP(tensor=i2.tensor, offset=i2.offset,
                                ap=[i2.ap[0], [0, 25], i2.ap[1]]),
                        bass.AP(tensor=wn.tensor, offset=wn.offset,
                                ap=[wn.ap[0], wn.ap[1], [0, B_IT]]),
                    )
            prev = (it, nt, wdiag)

            if dbg and it == 0:
                nc.gpsimd.dma_start(out=dbg_nt.ap(), in_=nt)
                nc.gpsimd.dma_start(out=dbg_ht.ap(), in_=ht)
                nc.gpsimd.dma_start(out=dbg_hh.ap(), in_=hh)
                nc.gpsimd.dma_start(out=dbg_w.ap(), in_=wnat)
                nc.gpsimd.dma_start(out=dbg_wd.ap(), in_=wdiag)

        if prev is not None and "pool" not in SKIP:
            emit_pool(prev)

    return out_pl


def decode_out(pl, dn, b_core=BC):
    """out_pl [nit, B_IT, D], out_dn [nit, 128, 2] -> pooled, wsum per b."""
    nit = b_core // B_IT
    pooled = pl.reshape(b_core, D)
    d = dn.reshape(nit, B_IT, 2, 2).astype(np.float64)
    wsum = d.sum(axis=(2, 3)).reshape(b_core).astype(np.float32)
    return pooled, wsum


_cache = {}
LAST_RESULT = None
SKIP = set()
CFG = dict(nt=3, tp=2, mm=2, q=1, pool=1, ht=2, hh=1, outt=2, wb=3, ntg=8)


def _get_program(b_core):
    key = (b_core, tuple(sorted(SKIP)), tuple(sorted(CFG.items())))
    if key not in _cache:
        nc = bacc.Bacc("TRN2", target_bir_lowering=False, debug=False,
                       num_devices=NCORES)
        build(nc, b_core)
        nc.compile()
        _cache[key] = nc
    return _cache[key]


def kernel(**inputs):
    hist = np.ascontiguousarray(np.asarray(inputs["hist_embeddings"], np.float32))
    tgt = np.ascontiguousarray(np.asarray(inputs["target_embedding"], np.float32))
    W = np.ascontiguousarray(np.asarray(inputs["W_kernel"], np.float32))
    Wb = np.ascontiguousarray(np.asarray(inputs["W_bias"], np.float32))
    q = np.ascontiguousarray(np.asarray(inputs["q_kernel"], np.float32))
    # q_bias shifts every logit equally -> softmax-invariant -> ignored.

    nc = _get_program(BC)
    in_maps = []
    for c in range(NCORES):
        sl = slice(c * BC, (c + 1) * BC)
        in_maps.append({
            "hist": hist[sl], "target": tgt[sl],
            "W": W, "Wb": Wb, "q": q,
        })
    res = run_bass_kernel_spmd(nc, in_maps, core_ids=list(range(NCORES)))
    global LAST_RESULT
    LAST_RESULT = res
    outs = []
    for c in range(NCORES):
        pooled, wsum = decode_out(res.results[c]["out_pl"],
                                  res.results[c]["out_dn"])
        outs.append(pooled / wsum[:, None])
    return np.concatenate(outs, axis=0).astype(np.float32)


def timed_run(inputs, iters=5, bcs=BC):
    """Device-resident repeated execution; returns (best_seconds, outputs)."""
    import time
    import jax
    from jax.sharding import Mesh, PartitionSpec
    from jax.experimental.shard_map import shard_map
    import concourse.mybir as mybir_
    from concourse.bass2jax import (install_neuronx_cc_hook, _bass_exec_p,
                                    partition_id_tensor)

    hist = np.ascontiguousarray(np.asarray(inputs["hist_embeddings"], np.float32))
    tgt = np.ascontiguousarray(np.asarray(inputs["target_embedding"], np.float32))
    W = np.ascontiguousarray(np.asarray(inputs["W_kernel"], np.float32))
    Wb = np.ascontiguousarray(np.asarray(inputs["W_bias"], np.float32))
    q = np.ascontiguousarray(np.asarray(inputs["q_kernel"], np.float32))
    hist = hist[:NCORES * bcs].reshape(NCORES * bcs, T, D)
    tgt = tgt[:NCORES * bcs]
    nc = _get_program(bcs)
    install_neuronx_cc_hook()

    pid_name = nc.partition_id_tensor.name if nc.partition_id_tensor else None
    in_names, out_names, out_avals, zero_outs = [], [], [], []
    for alloc in nc.m.functions[0].allocations:
        if not isinstance(alloc, mybir_.MemoryLocationSet):
            continue
        name = alloc.memorylocations[0].name
        if alloc.kind == "ExternalInput":
            if name != pid_name:
                in_names.append(name)
        elif alloc.kind == "ExternalOutput":
            shape = tuple(alloc.tensor_shape)
            dtype = mybir_.dt.np(alloc.dtype)
            out_names.append(name)
            out_avals.append(jax.core.ShapedArray(shape, dtype))
            zero_outs.append(np.zeros(shape, dtype))
    all_names = in_names + out_names
    if pid_name is not None:
        all_names = all_names + [pid_name]

    import os
    chain = int(os.environ.get("KERNEL_CHAIN", "1"))

    def _body(*args):
        nin_ = len(in_names)
        ins_ = list(args[:nin_])
        outs = list(args[nin_:])
        for _ in range(chain):
            operands = ins_ + outs
            if pid_name is not None:
                operands = operands + [partition_id_tensor()]
            outs = list(_bass_exec_p.bind(
                *operands, out_avals=tuple(out_avals),
                in_names=tuple(all_names), out_names=tuple(out_names),
                lowering_input_output_aliases=(),
                sim_require_finite=True, sim_require_nnan=True, nc=nc))
        return tuple(outs)

    devices = jax.devices()[:NCORES]
    mesh = Mesh(np.array(devices), ("core",))
    nin = len(in_names) + len(out_names)
    fn = jax.jit(shard_map(_body, mesh=mesh,
                           in_specs=(PartitionSpec("core"),) * nin,
                           out_specs=(PartitionSpec("core"),) * len(out_names),
                           check_rep=False))
    full = {"hist": hist, "target": tgt,
            "W": np.concatenate([W] * NCORES, 0),
            "Wb": np.concatenate([Wb] * NCORES, 0),
            "q": np.concatenate([q] * NCORES, 0)}
    args = [full[n] for n in in_names] + [
        np.concatenate([z] * NCORES, 0) for z in zero_outs]
    sh = jax.sharding.NamedSharding(mesh, PartitionSpec("core"))
    dargs = [jax.device_put(a, sh) for a in args]
    res = fn(*dargs)
    jax.block_until_ready(res)
    import os
    pipeline = int(os.environ.get("KERNEL_PIPE", "1"))
    nin_ = len(in_names)
    best = float("inf")
    for _ in range(iters):
        t0 = time.perf_counter()
        r = tuple(dargs[nin_:])
        for _k in range(pipeline):
            r = fn(*dargs[:nin_], *r)
        jax.block_until_ready(r)
        best = min(best, time.perf_counter() - t0)
        res = r
    outs = [np.asarray(r) for r in res]
    pl_all = np.split(outs[out_names.index("out_pl")], NCORES, axis=0)
    dn_all = np.split(outs[out_names.index("out_dn")], NCORES, axis=0)
    full_out = []
    for c in range(NCORES):
        pooled, wsum = decode_out(pl_all[c], dn_all[c], bcs)
        full_out.append(pooled / wsum[:, None])
    return best, np.concatenate(full_out, 0).astype(np.float32)


if __name__ == "__main__":
    rng = np.random.default_rng(0)
    ins = {
        "target_embedding": rng.standard_normal((B, D), dtype=np.float32),
        "hist_embeddings": rng.standard_normal((B, T, D), dtype=np.float32),
        "W_kernel": (rng.standard_normal((D, D), dtype=np.float32) / np.sqrt(D)),
        "W_bias": np.zeros(D, np.float32),
        "q_kernel": (rng.standard_normal((D, 1), dtype=np.float32) / np.sqrt(D)),
        "q_bias": np.zeros(1, np.float32),
    }
    out = kernel(**ins)
    print("out", out.shape, out.dtype)


# revision 36
# speedup vs baseline: 1.5693x; 1.1105x over previous
"""AttentionPooling Trainium2 kernel (v3: contiguous load + diagonal pooling).

Math (per batch row b):
    x   = target[b] + hist[b]              # [T, D]
    h   = relu(x @ W + Wb)                 # [T, D]
    lg  = h @ q (+ q_bias, softmax-invariant -> ignored)
    s   = softmax(lg)                      # over T
    out = sum_t s_t * hist[b, t]           # [D]

Device strategy (pure data parallel over batch across 8 cores).  Strided
HBM reads run at ~half bandwidth on real TRN2, so hist is loaded with a
single fully CONTIGUOUS fp32->bf16 cast DMA per 64-batch iteration into
the natural layout [p=(b,th), (tl,d)] (th = t//100, tl = t%100):
  - PE transposes the 100 [128,128] d-blocks -> xT [d, (tl, p)]; the
    PSUM->SBUF copy fuses the broadcast target add (packed APs keep the
    DVE 2x bf16 mode).
  - Main matmul: H^T = W^T @ xT (bf16, W stationary), relu+bias on ACT
    over 1024-col PSUM chunks.
  - q-matmul per tl-chunk: stationary = hh block [e,128], moving = q
    -> logits land NATURALLY as [p, tl] columns of one [128,100] PSUM
    tile; a single exp (ACT) with accum_out yields w AND the softmax
    denominators in one instruction.
  - Pooling: per tl one matmul, stationary = wdiag [128, 64] (w values
    scattered on the 2-diagonal (p, p//2), built by one DVE multiply
    with a constant 0/1 mask), moving = the natural hist block
    [128, 128] -> PSUM-accumulated [b, d] over all 100 tl.
  - Final normalize (divide by sum_t w) on host.
"""

import sys

sys.path.insert(0, "/opt/trn_rl_repo")

import numpy as np

import concourse.bacc as bacc
import concourse.bass as bass
import concourse.mybir as mybir
import concourse.tile as tile
from concourse import masks
from concourse.bass_utils import run_bass_kernel_spmd

F32 = mybir.dt.float32
BF16 = mybir.dt.bfloat16
AF = mybir.ActivationFunctionType

NCORES = 8
B, T, D = 16384, 200, 128
BC = B // NCORES          # 2048 batch rows per core
TL = 100                  # tl positions per partition (t = th*100 + tl)
B_IT = 64                 # batch rows per outer iteration
NC_IT = B_IT * T * D      # elements per iteration


def build(nc, b_core=BC, dbg=False):
    nit = b_core // B_IT
    hist = nc.dram_tensor("hist", [b_core, T, D], F32, kind="ExternalInput")
    tgt = nc.dram_tensor("target", [b_core, D], F32, kind="ExternalInput")
    w_in = nc.dram_tensor("W", [D, D], F32, kind="ExternalInput")
    wb_in = nc.dram_tensor("Wb", [D], F32, kind="ExternalInput")
    q_in = nc.dram_tensor("q", [D, 1], F32, kind="ExternalInput")
    out_pl = nc.dram_tensor("out_pl", [nit, B_IT, D], F32, kind="ExternalOutput")
    out_dn = nc.dram_tensor("out_dn", [nit, 128, 2], F32, kind="ExternalOutput")
    if dbg:
        dbg_nt = nc.dram_tensor("dbg_nt", [128, TL * D], F32, kind="ExternalOutput")
        dbg_ht = nc.dram_tensor("dbg_ht", [128, B_IT * T], F32, kind="ExternalOutput")
        dbg_hh = nc.dram_tensor("dbg_hh", [128, B_IT * T], F32, kind="ExternalOutput")
        dbg_w = nc.dram_tensor("dbg_w", [128, TL], F32, kind="ExternalOutput")
        dbg_wd = nc.dram_tensor("dbg_wd", [128, TL * B_IT], F32, kind="ExternalOutput")

    from contextlib import ExitStack
    with tile.TileContext(nc) as tc, ExitStack() as es:
        consts = es.enter_context(tc.tile_pool(name="consts", bufs=1))
        nt_pool = es.enter_context(tc.tile_pool(name="nt", bufs=CFG["nt"]))
        ht_pool = es.enter_context(tc.tile_pool(name="ht", bufs=CFG["ht"]))
        h_pool = es.enter_context(tc.tile_pool(name="h", bufs=CFG["hh"]))
        w_pool = es.enter_context(tc.tile_pool(name="w", bufs=CFG["wb"]))
        out_pool = es.enter_context(tc.tile_pool(name="out", bufs=CFG["outt"]))
        ps_tp = es.enter_context(tc.tile_pool(name="ps_tp", bufs=CFG["tp"], space="PSUM"))
        ps_mm = es.enter_context(tc.tile_pool(name="ps_mm", bufs=CFG["mm"], space="PSUM"))
        ps_q = es.enter_context(tc.tile_pool(name="ps_q", bufs=CFG["q"], space="PSUM"))
        ps_pool = es.enter_context(tc.tile_pool(name="ps_pool", bufs=CFG["pool"], space="PSUM"))

        # ---- constants ----
        ident = consts.tile([128, 128], BF16)
        masks.make_identity(nc, ident[:, :])

        w_f32 = consts.tile([D, D], F32)
        nc.sync.dma_start(out=w_f32, in_=w_in.ap())
        w_bf = consts.tile([D, D], BF16)
        nc.vector.tensor_copy(out=w_bf, in_=w_f32)

        wbias = consts.tile([D, 1], F32)
        nc.sync.dma_start(out=wbias, in_=wb_in.ap()[:, None])

        q_f32 = consts.tile([D, 1], F32)
        nc.sync.dma_start(out=q_f32, in_=q_in.ap())
        q_bf = consts.tile([D, 1], BF16)
        nc.vector.tensor_copy(out=q_bf, in_=q_f32)

        # 2-diagonal mask: I2[p, b] = 1 if p // 2 == b else 0  [128, 64] bf16
        # built from the identity: I2[p, b] = ident[p, 2b] + ident[p, 2b+1]
        i2 = consts.tile([128, B_IT], BF16)
        idv = ident.rearrange("p (b u) -> p b u", u=2)
        nc.vector.tensor_add(i2, idv[:, :, 0], idv[:, :, 1])

        # targetT [d, b_core] bf16
        tgtT = consts.tile([D, b_core], BF16)
        for k in range((b_core + 127) // 128):
            bn = min(128, b_core - k * 128)
            t_f32 = w_pool.tile([128, D], F32, tag="tsetup")
            nc.sync.dma_start(out=t_f32[0:bn], in_=tgt.ap()[k * 128:k * 128 + bn, :])
            t_bf = w_pool.tile([128, D], BF16, tag="tsetup_bf")
            nc.vector.tensor_copy(out=t_bf[0:bn], in_=t_f32[0:bn])
            tpp = ps_tp.tile([128, 1024], BF16, tag="tp")
            nc.tensor.transpose(tpp[:, 0:bn], t_bf[0:bn], ident[0:bn, 0:bn])
            nc.vector.tensor_copy(out=tgtT[:, k * 128:k * 128 + bn], in_=tpp[:, 0:bn])

        # ---- main loop ----
        # Pooling for iteration it is emitted during iteration it+1 (after
        # the transposes) so the PE never stalls on the exp -> wdiag chain.
        def emit_pool(prev):
            it_p, nt_p, wd_p = prev
            pl = ps_pool.tile([B_IT, D], F32)
            for tl in range(TL):
                nc.tensor.matmul(pl, wd_p[:, tl * B_IT:(tl + 1) * B_IT],
                                 nt_p[:, tl * D:tl * D + D],
                                 start=tl == 0, stop=tl == TL - 1)
            outt = out_pool.tile([B_IT, D], F32, tag="outt")
            nc.vector.tensor_copy(out=outt, in_=pl)
            nc.sync.dma_start(out=out_pl.ap()[it_p], in_=outt)

        prev = None
        for it in range(nit):
            b0 = it * B_IT

            # natural tile: partition p=(b,th), free (tl, d); one contiguous
            # cast DMA for the whole 64-batch slice
            nt = nt_pool.tile([128, TL * D], BF16, tag="nt")
            src = hist.ap()[b0:b0 + B_IT]
            nds = CFG.get("dmas", 1)   # DMA split count (free-dim chunks)
            half = TL * D // nds
            for c in range(nds):
                nc.gpsimd.dma_start(
                    out=nt[:, c * half:(c + 1) * half],
                    in_=bass.AP(tensor=src.tensor,
                                offset=src.offset + c * half,
                                ap=[[TL * D, 128], [1, half]]),
                )

            # target expanded 2x: tgx2[d, p] = tgtT[d, b0 + p//2] -- i.e.
            # column index IS p = 2b+th, so the broadcast AP below has a
            # fully packed innermost dim (DVE 2x mode)
            tgx2 = w_pool.tile([128, B_IT * 2], BF16, tag="tgx2")
            sl = tgtT[:, b0:b0 + B_IT]
            nc.vector.tensor_copy(
                out=tgx2,
                in_=bass.AP(tensor=sl.tensor, offset=sl.offset,
                            ap=[sl.ap[0], sl.ap[1], [0, 2]]),
            )

            # transposes -> xT [d, (tl, p)] with fused target add
            ht = ht_pool.tile([128, B_IT * T], BF16, tag="ht")
            NTG = CFG["ntg"]          # transposes per PSUM group (8 -> 1 bank)
            for g in range((TL + NTG - 1) // NTG) if "tp" not in SKIP else []:
                t0 = NTG * g
                ng = min(NTG, TL - t0)
                tp = ps_tp.tile([128, NTG * 128], BF16, tag="tp")
                for u in range(ng):
                    nc.tensor.transpose(
                        tp[:, 128 * u:128 * u + 128],
                        nt[:, (t0 + u) * D:(t0 + u) * D + D], ident)
                nc.vector.tensor_add(
                    ht.rearrange("d (t p) -> d t p",
                                 p=128)[:, t0:t0 + ng, :],
                    tp.rearrange("d (t p) -> d t p", p=128)[:, 0:ng, :],
                    bass.AP(tensor=tgx2.tensor, offset=tgx2.offset,
                            ap=[tgx2.ap[0], [0, ng], [1, 128]]),
                )

            if prev is not None and "pool" not in SKIP and not CFG.get("ilv"):
                emit_pool(prev)

            # H^T = relu(W^T xT + bias)  [e, (tl, p)]
            hh = h_pool.tile([128, B_IT * T], BF16, tag="hh")
            nmm = (B_IT * T) // 1024
            for k in range(nmm + 1) if "mm" not in SKIP else []:
                w_cols = 1024 if k < nmm else (B_IT * T) % 1024
                if w_cols == 0:
                    continue
                mm = ps_mm.tile([128, 1024], F32)
                for h in range((w_cols + 511) // 512):
                    c = 1024 * k + 512 * h
                    cw = min(512, w_cols - 512 * h)
                    nc.tensor.matmul(mm[:, 512 * h:512 * h + cw], w_bf,
                                     ht[:, c:c + cw], start=True, stop=True)
                nc.scalar.activation(hh[:, 1024 * k:1024 * k + w_cols],
                                     mm[:, 0:w_cols], AF.Relu, bias=wbias)

            # q-matmuls: logits land naturally [p, tl].  With CFG["ilv"],
            # the previous iteration's pool matmuls are interleaved so the
            # qn stationary loads hide under their 128-col streams.
            qn = ps_q.tile([128, TL], F32)
            do_ilv = (CFG.get("ilv") and prev is not None
                      and "pool" not in SKIP)
            if do_ilv:
                it_p, nt_p, wd_p = prev
                pl = ps_pool.tile([B_IT, D], F32)
            for tl in range(TL) if "q" not in SKIP else []:
                nc.tensor.matmul(qn[:, tl:tl + 1],
                                 hh[:, tl * 128:tl * 128 + 128], q_bf,
                                 start=True, stop=True)
                if do_ilv:
                    nc.tensor.matmul(pl, wd_p[:, tl * B_IT:(tl + 1) * B_IT],
                                     nt_p[:, tl * D:tl * D + D],
                                     start=tl == 0, stop=tl == TL - 1)
            if do_ilv:
                outt = out_pool.tile([B_IT, D], F32, tag="outt")
                nc.vector.tensor_copy(out=outt, in_=pl)
                nc.sync.dma_start(out=out_pl.ap()[it_p], in_=outt)

            # exp in 2 chunks (pipelines with qn); accum gives denominators
            wnat = w_pool.tile([128, TL], BF16, tag="wnat")
            dn_sb = out_pool.tile([128, 2], F32, tag="dn")
            if "q" not in SKIP:
                for c in range(2):
                    nc.scalar.activation(wnat[:, 50 * c:50 * c + 50],
                                         qn[:, 50 * c:50 * c + 50], AF.Exp,
                                         accum_out=dn_sb[:, c:c + 1])
                nc.sync.dma_start(out=out_dn.ap()[it], in_=dn_sb)

            # wdiag build in 4 chunks: wdiag[p, (tl, b)] = I2[p, b] * wnat[p, tl]
            wdiag = w_pool.tile([128, TL * B_IT], BF16, tag="wdiag")
            wdv = wdiag.rearrange("p (t b) -> p t b", b=B_IT)
            if "pool" not in SKIP and "q" not in SKIP:
                for c in range(4):
                    wn = wnat[:, 25 * c:25 * c + 25]
                    nc.vector.tensor_mul(
                        wdv[:, 25 * c:25 * c + 25, :],
                        bass.AP(tensor=i2.tensor, offset=i2.offset,
                                ap=[i2.ap[0], [0, 25], i2.ap[1]]),
                        bass.AP(tensor=wn.tensor, offset=wn.offset,
                                ap=[wn.ap[0], wn.ap[1], [0, B_IT]]),
                    )
            prev = (it, nt, wdiag)

            if dbg and it == 0:
                nc.gpsimd.dma_start(out=dbg_nt.ap(), in_=nt)
                nc.gpsimd.dma_start(out=dbg_ht.ap(), in_=ht)
                nc.gpsimd.dma_start(out=dbg_hh.ap(), in_=hh)
                nc.gpsimd.dma_start(out=dbg_w.ap(), in_=wnat)
                nc.gpsimd.dma_start(out=dbg_wd.ap(), in_=wdiag)

        if prev is not None and "pool" not in SKIP:
            emit_pool(prev)

    return out_pl


def decode_out(pl, dn, b_core=BC):
    """out_pl [nit, B_IT, D], out_dn [nit, 128, 2] -> pooled, wsum per b."""
    nit = b_core // B_IT
    pooled = pl.reshape(b_core, D)
    d = dn.reshape(nit, B_IT, 2, 2).astype(np.float64)
    wsum = d.sum(axis=(2, 3)).reshape(b_core).astype(np.float32)
    return pooled, wsum


_cache = {}
LAST_RESULT = None
SKIP = set()
CFG = dict(nt=3, tp=2, mm=2, q=1, pool=1, ht=2, hh=1, outt=2, wb=3, ntg=8)


def _get_program(b_core):
    key = (b_core, tuple(sorted(SKIP)), tuple(sorted(CFG.items())))
    if key not in _cache:
        nc = bacc.Bacc("TRN2", target_bir_lowering=False, debug=False,
                       num_devices=NCORES)
        build(nc, b_core)
        nc.compile()
        _cache[key] = nc
    return _cache[key]


def kernel(**inputs):
    hist = np.ascontiguousarray(np.asarray(inputs["hist_embeddings"], np.float32))
    tgt = np.ascontiguousarray(np.asarray(inputs["target_embedding"], np.float32))
    W = np.ascontiguousarray(np.asarray(inputs["W_kernel"], np.float32))
    Wb = np.ascontiguousarray(np.asarray(inputs["W_bias"], np.float32))
    q = np.ascontiguousarray(np.asarray(inputs["q_kernel"], np.float32))
    # q_bias shifts every logit equally -> softmax-invariant -> ignored.

    nc = _get_program(BC)
    in_maps = []
    for c in range(NCORES):
        sl = slice(c * BC, (c + 1) * BC)
        in_maps.append({
            "hist": hist[sl], "target": tgt[sl],
            "W": W, "Wb": Wb, "q": q,
        })
    res = run_bass_kernel_spmd(nc, in_maps, core_ids=list(range(NCORES)))
    global LAST_RESULT
    LAST_RESULT = res
    outs = []
    for c in range(NCORES):
        pooled, wsum = decode_out(res.results[c]["out_pl"],
                                  res.results[c]["out_dn"])
        outs.append(pooled / wsum[:, None])
    return np.concatenate(outs, axis=0).astype(np.float32)


def timed_run(inputs, iters=5, bcs=BC):
    """Device-resident repeated execution; returns (best_seconds, outputs)."""
    import time
    import jax
    from jax.sharding import Mesh, PartitionSpec
    from jax.experimental.shard_map import shard_map
    import concourse.mybir as mybir_
    from concourse.bass2jax import (install_neuronx_cc_hook, _bass_exec_p,
                                    partition_id_tensor)

    hist = np.ascontiguousarray(np.asarray(inputs["hist_embeddings"], np.float32))
    tgt = np.ascontiguousarray(np.asarray(inputs["target_embedding"], np.float32))
    W = np.ascontiguousarray(np.asarray(inputs["W_kernel"], np.float32))
    Wb = np.ascontiguousarray(np.asarray(inputs["W_bias"], np.float32))
    q = np.ascontiguousarray(np.asarray(inputs["q_kernel"], np.float32))
    hist = hist[:NCORES * bcs].reshape(NCORES * bcs, T, D)
    tgt = tgt[:NCORES * bcs]
    nc = _get_program(bcs)
    install_neuronx_cc_hook()

    pid_name = nc.partition_id_tensor.name if nc.partition_id_tensor else None
    in_names, out_names, out_avals, zero_outs = [], [], [], []
    for alloc in nc.m.functions[0].allocations:
        if not isinstance(alloc, mybir_.MemoryLocationSet):
            continue
        name = alloc.memorylocations[0].name
        if alloc.kind == "ExternalInput":
            if name != pid_name:
                in_names.append(name)
        elif alloc.kind == "ExternalOutput":
            shape = tuple(alloc.tensor_shape)
            dtype = mybir_.dt.np(alloc.dtype)
            out_names.append(name)
            out_avals.append(jax.core.ShapedArray(shape, dtype))
            zero_outs.append(np.zeros(shape, dtype))
    all_names = in_names + out_names
    if pid_name is not None:
        all_names = all_names + [pid_name]

    import os
    chain = int(os.environ.get("KERNEL_CHAIN", "1"))

    def _body(*args):
        nin_ = len(in_names)
        ins_ = list(args[:nin_])
        outs = list(args[nin_:])
        for _ in range(chain):
            operands = ins_ + outs
            if pid_name is not None:
                operands = operands + [partition_id_tensor()]
            outs = list(_bass_exec_p.bind(
                *operands, out_avals=tuple(out_avals),
                in_names=tuple(all_names), out_names=tuple(out_names),
                lowering_input_output_aliases=(),
                sim_require_finite=True, sim_require_nnan=True, nc=nc))
        return tuple(outs)

    devices = jax.devices()[:NCORES]
    mesh = Mesh(np.array(devices), ("core",))
    nin = len(in_names) + len(out_names)
    fn = jax.jit(shard_map(_body, mesh=mesh,
                           in_specs=(PartitionSpec("core"),) * nin,
                           out_specs=(PartitionSpec("core"),) * len(out_names),
                           check_rep=False))
    full = {"hist": hist, "target": tgt,
            "W": np.concatenate([W] * NCORES, 0),
            "Wb": np.concatenate([Wb] * NCORES, 0),
            "q": np.concatenate([q] * NCORES, 0)}
    args = [full[n] for n in in_names] + [
        np.concatenate([z] * NCORES, 0) for z in zero_outs]
    sh = jax.sharding.NamedSharding(mesh, PartitionSpec("core"))
    dargs = [jax.device_put(a, sh) for a in args]
    res = fn(*dargs)
    jax.block_until_ready(res)
    import os
    pipeline = int(os.environ.get("KERNEL_PIPE", "1"))
    nin_ = len(in_names)
    best = float("inf")
    for _ in range(iters):
        t0 = time.perf_counter()
        r = tuple(dargs[nin_:])
        for _k in range(pipeline):
            r = fn(*dargs[:nin_], *r)
        jax.block_until_ready(r)
        best = min(best, time.perf_counter() - t0)
        res = r
    outs = [np.asarray(r) for r in res]
    pl_all = np.split(outs[out_names.index("out_pl")], NCORES, axis=0)
    dn_all = np.split(outs[out_names.index("out_dn")], NCORES, axis=0)
    full_out = []
    for c in range(NCORES):
        pooled, wsum = decode_out(pl_all[c], dn_all[c], bcs)
        full_out.append(pooled / wsum[:, None])
    return best, np.concatenate(full_out, 0).astype(np.float32)


if __name__ == "__main__":
    rng = np.random.default_rng(0)
    ins = {
        "target_embedding": rng.standard_normal((B, D), dtype=np.float32),
        "hist_embeddings": rng.standard_normal((B, T, D), dtype=np.float32),
        "W_kernel": (rng.standard_normal((D, D), dtype=np.float32) / np.sqrt(D)),
        "W_bias": np.zeros(D, np.float32),
        "q_kernel": (rng.standard_normal((D, 1), dtype=np.float32) / np.sqrt(D)),
        "q_bias": np.zeros(1, np.float32),
    }
    out = kernel(**ins)
    print("out", out.shape, out.dtype)
